# revision 1
# baseline (speedup 1.0000x reference)
"""MinGRU (L=2, B=8, S=2048, D=H=1024) Trainium2 Bass kernel.

Sharding: data-parallel over batch B across the 8 NeuronCores (1 sequence
per core); the (L,H,D) weights are replicated.

Per-core dataflow (all shapes per core):
  inputs (host-preprocessed): xT (D,S) fp16, WzT/WhT (L,D,H) fp16,
  biases as (L,4,128,H/128) fp32 tiles (bz, -bz, bh, bh+0.5).
  layer l:
    k  = Wz_l @ x          -> PSUM (h-part, s-free), 8 accumulating matmuls
    p  = Wh_l @ x          -> PSUM
    z  = sigmoid(k + bz)          (ACT, PSUM->SBUF)
    c  = sigmoid(-(k + bz))       (ACT, scale=-1, bias=-bz)
    sg = sigmoid(p + bh)          (ACT)
    g  = max(p + bh + 0.5, sg)    (DVE scalar_tensor_tensor; exact identity
                                   for MinGRU's piecewise g)
    v  = z * g                    (DVE)
    h[t] = c[t]*h[t-1] + v[t], h0=0.5   (DVE tensor_tensor_scan, fp32 state)
  layer-0 scan emits fp16 directly into the layer-1 rhs buffer (the scan
  layout IS the next layer's matmul rhs layout — no device transposes);
  layer-1 scan emits fp32 chunks DMA'd to DRAM as (H,S); the host
  transposes back to (S,H). The linear-space scan is numerically safe:
  all terms positive, h bounded in [~1e-3, ~4].
"""

import os
import sys

for _p in (
    "/root/.axon_site",
    "/root/.axon_site/_ro/trn_rl_repo",
    "/root/.axon_site/_ro/pypackages",
    "/opt/trn_rl_repo",
    "/opt/pypackages",
):
    if os.path.isdir(_p) and _p not in sys.path:
        sys.path.append(_p)

from contextlib import ExitStack

import numpy as np

import concourse.bacc as bacc
import concourse.bass as bass
import concourse.tile as tile
from concourse import mybir

L, B, S, D, H = 2, 8, 2048, 1024, 1024
P = 128
DT = D // P          # 8 contraction tiles
HT = H // P          # 8 output-channel tiles
SB = 512             # time-block (one PSUM bank of fp32)
NSB = S // SB        # 4

F16 = mybir.dt.float16
F32 = mybir.dt.float32
AF = mybir.ActivationFunctionType
OP = mybir.AluOpType

LAST_EXEC_NS = None

_BUILT = None


def _build(reps=1, mm_only=False, sb=SB, k_first=False):
    global SB, NSB
    SB, NSB = sb, S // sb
    nc = bacc.Bacc("TRN2", target_bir_lowering=False, debug=False)

    xT = nc.dram_tensor("xT", (D, S), F16, kind="ExternalInput")
    wzT = nc.dram_tensor("wzT", (L, D, H), F16, kind="ExternalInput")
    whT = nc.dram_tensor("whT", (L, D, H), F16, kind="ExternalInput")
    # biases pre-tiled on host: [l, f, p, ht] = bias_f[l, ht*128 + p]
    # f in (bz, -bz, bh, bh+0.5)
    bias_d = nc.dram_tensor("biases", (L, 4, P, HT), F32, kind="ExternalInput")
    outT = nc.dram_tensor("outT", (H, S), F32, kind="ExternalOutput")

    xT_r = xT.rearrange("(dt p) s -> p dt s", p=P)

    with tile.TileContext(nc) as tc, ExitStack() as ctx:
        persist = ctx.enter_context(tc.tile_pool(name="persist", bufs=1))
        cvpool = ctx.enter_context(tc.tile_pool(name="cv", bufs=3))
        zpool = ctx.enter_context(tc.tile_pool(name="zs", bufs=3))
        # layer-1 output chunks: chain distance between same-ht chunks is
        # HT units (x sub-chunks) in sb-major order; keep enough slots live
        ochunk_pool = ctx.enter_context(tc.tile_pool(name="ochunk", bufs=2 * HT + 2))
        pk_pool = ctx.enter_context(tc.tile_pool(name="pk", bufs=2, space="PSUM"))
        pp_pool = ctx.enter_context(tc.tile_pool(name="pp", bufs=2, space="PSUM"))
        warm_pool = ctx.enter_context(tc.tile_pool(name="warm", bufs=1, space="PSUM"))

        # ---- persistent SBUF state ----
        x_sb = persist.tile([P, DT, S], F16)       # layer-0 input (xT)
        h1_sb = persist.tile([P, HT, S], F16)      # layer-0 output = layer-1 rhs
        w_sb = {}
        for l in range(L):
            for nm, dram in (("wz", wzT), ("wh", whT)):
                w_sb[(nm, l)] = persist.tile([P, DT, H], F16, name=f"{nm}{l}_sb")
        bias_tiles = [
            persist.tile([P, 4, HT], F32, name=f"bias{l}_sb") for l in range(L)
        ]
        bias_sb = {}
        for l in range(L):
            for fi, nm in enumerate(("bz", "bzn", "bh", "bh05")):
                bias_sb[(nm, l)] = bias_tiles[l][:, fi]

        def load_w(nm, l, h0, h1):
            # one 3D-AP DMA per slice: per-partition DT chunks of (h1-h0)
            src = {"wz": wzT, "wh": whT}[nm][l].rearrange("(dt p) h -> p dt h", p=P)
            nc.sync.dma_start(out=w_sb[(nm, l)][:, :, h0:h1], in_=src[:, :, h0:h1])

        def load_x(sb):
            nc.sync.dma_start(
                out=x_sb[:, :, sb * SB : (sb + 1) * SB],
                in_=xT_r[:, :, sb * SB : (sb + 1) * SB],
            )

        # PE warmup: dummy matmuls on a zeroed tile run during the DMA
        # lead-in so the HAM clock gate reaches 2.4 GHz before real work.
        # 41 x 213ns ~= 8.7us, sized so PE does not outpace the input DMA
        # stream (fewer warmups create a mid-stream stall that also drops
        # the clock ramp).
        warm = persist.tile([P, SB], F16, name="warm")
        warm_ps = warm_pool.tile([P, SB], F32, name="warm_ps")
        nc.vector.memset(warm, 0.0)
        for _ in range(41):
            nc.tensor.matmul(warm_ps, warm[:, :P], warm, start=True, stop=True)

        # DMA emission in first-consumption order, minimizing DMA count on
        # the critical path (per-DMA queue overhead is significant).
        load_w("wz", 0, 0, P)        # 0.25 MB — first unit's k weights
        load_w("wh", 0, 0, P)        # 0.25 MB — first unit's p weights
        load_x(0)                    # 1 MB
        load_w("wz", 0, P, H // 2)
        load_w("wh", 0, P, H // 2)
        load_w("wz", 0, H // 2, H)
        load_w("wh", 0, H // 2, H)
        nc.sync.dma_start(
            out=bias_tiles[0], in_=bias_d[0].rearrange("f p ht -> p f ht")
        )
        for sb in range(1, NSB):
            load_x(sb)
        nc.sync.dma_start(
            out=bias_tiles[1], in_=bias_d[1].rearrange("f p ht -> p f ht")
        )
        for half in range(2):
            load_w("wz", 1, half * (H // 2), (half + 1) * (H // 2))
            load_w("wh", 1, half * (H // 2), (half + 1) * (H // 2))

        def layer(l, rhs_sb, out_writer, split_last=False):
            """rhs_sb: [P, DT, S] f16 input; out_writer(ht, col0, w, c, v)."""
            wz = w_sb[("wz", l)]
            wh = w_sb[("wh", l)]
            bz_t = bias_sb[("bz", l)]
            bzn_t = bias_sb[("bzn", l)]
            bh_t = bias_sb[("bh", l)]
            bh05_t = bias_sb[("bh05", l)]
            for sb in range(NSB):
                s0, s1 = sb * SB, (sb + 1) * SB
                for ht in range(HT):
                    h0, h1 = ht * P, (ht + 1) * P
                    pk = pk_pool.tile([P, SB], F32, name="pk")
                    pp = pp_pool.tile([P, SB], F32, name="pp")
                    # p-group first: sg/g depend only on p, so they overlap
                    # the k-group's matmuls — shortens each unit's epilogue
                    # and the kernel tail
                    groups = [(pp, wh), (pk, wz)]
                    if k_first:
                        groups = [(pk, wz), (pp, wh)]
                    for ps, wmat in groups:
                        for dt_i in range(DT):
                            nc.tensor.matmul(
                                ps,
                                wmat[:, dt_i, h0:h1],
                                rhs_sb[:, dt_i, s0:s1],
                                start=(dt_i == 0),
                                stop=(dt_i == DT - 1),
                            )
                    if mm_only:
                        continue
                    # shorten the kernel tail: the very last unit's epilogue
                    # runs after the last matmul, so process it in two halves
                    sub = 2 if (split_last and sb == NSB - 1 and ht == HT - 1) else 1
                    w = SB // sub
                    for si in range(sub):
                        c0, c1 = si * w, (si + 1) * w
                        z = zpool.tile([P, SB], F32, name="z")[:, :w]
                        sg = zpool.tile([P, SB], F32, name="sg")[:, :w]
                        g = zpool.tile([P, SB], F32, name="g")[:, :w]
                        c = cvpool.tile([P, SB], F32, name="c")[:, :w]
                        v = cvpool.tile([P, SB], F32, name="v")[:, :w]
                        # ACT order matches the DVE dependency chain: g needs
                        # sg first; the scan needs c last — shortens the
                        # ACT->DVE critical path of each unit (and the tail)
                        nc.scalar.activation(
                            sg, pp[:, c0:c1], AF.Sigmoid,
                            bias=bh_t[:, ht : ht + 1], scale=1.0,
                        )
                        nc.scalar.activation(
                            z, pk[:, c0:c1], AF.Sigmoid,
                            bias=bz_t[:, ht : ht + 1], scale=1.0,
                        )
                        nc.scalar.activation(
                            c, pk[:, c0:c1], AF.Sigmoid,
                            bias=bzn_t[:, ht : ht + 1], scale=-1.0,
                        )
                        # g = (p + (bh+0.5)) max sigmoid(p+bh)
                        nc.vector.scalar_tensor_tensor(
                            g, pp[:, c0:c1], bh05_t[:, ht : ht + 1], sg,
                            op0=OP.add, op1=OP.max,
                        )
                        nc.vector.tensor_mul(v, z, g)
                        out_writer(ht, s0 + c0, w, c, v)

        # layer 0: scan into h1_sb (f16), chained across blocks
        def l0_writer(ht, col0, w, c, v):
            dst = h1_sb[:, ht, col0 : col0 + w]
            init = 0.5 if col0 == 0 else h1_sb[:, ht, col0 - 1 : col0]
            nc.vector.tensor_tensor_scan(dst, c, v, init, op0=OP.mult, op1=OP.add)

        # layer 1: scan into fp32 chunks, DMA out per chunk
        prev_chunk = {}

        def l1_writer(ht, col0, w, c, v):
            oc = ochunk_pool.tile([P, SB], F32, name="oc")[:, :w]
            if col0 == 0:
                init = 0.5
            else:
                pt, pw = prev_chunk[ht]
                init = pt[:, pw - 1 : pw]
            nc.vector.tensor_tensor_scan(oc, c, v, init, op0=OP.mult, op1=OP.add)
            prev_chunk[ht] = (oc, w)
            nc.sync.dma_start(out=outT[ht * P : (ht + 1) * P, col0 : col0 + w], in_=oc)

        def body():
            layer(0, x_sb, l0_writer)
            layer(1, x_sb if mm_only else h1_sb, l1_writer, split_last=True)

        if reps == 1:
            body()
        else:
            # timing-only: run the body `reps` times in a hardware loop so
            # one dispatch amortizes the host->terminal RPC floor
            with tc.For_i(0, reps, 1, hint_engines=tuple(nc.engines)):
                body()

    nc.finalize()
    return nc


class _Runner:
    """Compile the bass module once into a jitted shard_map over 8 cores."""

    def __init__(self, reps=1, mm_only=False, sb=512, k_first=False):
        import jax
        from jax.experimental.shard_map import shard_map
        from jax.sharding import Mesh, NamedSharding, PartitionSpec

        from concourse import bass2jax, mybir as _mybir

        self.jax = jax
        nc = _build(reps, mm_only=mm_only, sb=sb, k_first=k_first)
        self.nc = nc
        bass2jax.install_neuronx_cc_hook()

        partition_name = (
            nc.partition_id_tensor.name if nc.partition_id_tensor else None
        )
        in_names, out_names, out_avals, zero_shapes = [], [], [], []
        for alloc in nc.m.functions[0].allocations:
            if not isinstance(_mybir.MemoryLocationSet, type) or not isinstance(
                alloc, _mybir.MemoryLocationSet
            ):
                continue
            name = alloc.memorylocations[0].name
            if alloc.kind == "ExternalInput":
                if name != partition_name:
                    in_names.append(name)
            elif alloc.kind == "ExternalOutput":
                shape = tuple(alloc.tensor_shape)
                dtype = _mybir.dt.np(alloc.dtype)
                out_names.append(name)
                out_avals.append(jax.core.ShapedArray(shape, dtype))
                zero_shapes.append((shape, dtype))
        self.in_names = list(in_names)
        self.out_names = out_names
        self.zero_shapes = zero_shapes
        n_params = len(in_names)
        n_outs = len(out_names)
        all_in_names = in_names + out_names
        if partition_name is not None:
            all_in_names.append(partition_name)
        donate = tuple(range(n_params, n_params + n_outs))

        def _body(*args):
            operands = list(args)
            if partition_name is not None:
                operands.append(bass2jax.partition_id_tensor())
            outs = bass2jax._bass_exec_p.bind(
                *operands,
                out_avals=tuple(out_avals),
                in_names=tuple(all_in_names),
                out_names=tuple(out_names),
                lowering_input_output_aliases=(),
                sim_require_finite=True,
                sim_require_nnan=True,
                nc=nc,
            )
            return tuple(outs)

        self._base_body = _body
        devices = jax.devices()[:B]
        assert len(devices) == B
        self.mesh = Mesh(np.asarray(devices), ("core",))
        self.sharding = NamedSharding(self.mesh, PartitionSpec("core"))
        in_specs = (PartitionSpec("core"),) * (n_params + n_outs)
        out_specs = (PartitionSpec("core"),) * n_outs
        _mapped = shard_map(
            _body,
            mesh=self.mesh,
            in_specs=in_specs,
            out_specs=out_specs,
            check_rep=False,
        )
        self.fn = jax.jit(_mapped, donate_argnums=donate, keep_unused=True)
        self.fn_nodonate = jax.jit(_mapped, keep_unused=True)

    def _concat_inputs(self, in_maps):
        return [
            np.concatenate([np.asarray(m[name]) for m in in_maps], axis=0)
            for name in self.in_names
        ]

    def _zeros(self):
        return [
            np.zeros((B * s[0], *s[1:]), dt) for (s, dt) in self.zero_shapes
        ]

    def run(self, in_maps):
        out_arrs = self.fn(*self._concat_inputs(in_maps), *self._zeros())
        return [
            {
                name: np.asarray(out_arrs[i]).reshape(B, -1, *out_arrs[i].shape[1:])[c]
                for i, name in enumerate(self.out_names)
            }
            for c in range(B)
        ]

    def bench_loop(self, in_maps, iters=16, inner=4):
        """Min wall time of `inner` back-to-back non-donating executions."""
        import time as _time

        jax = self.jax
        dev_in = [
            jax.device_put(a, self.sharding) for a in self._concat_inputs(in_maps)
        ]
        dev_z = [jax.device_put(z, self.sharding) for z in self._zeros()]
        out = self.fn_nodonate(*dev_in, *dev_z)
        jax.block_until_ready(out)
        best = float("inf")
        for _ in range(iters):
            t0 = _time.perf_counter()
            for _ in range(inner):
                out = self.fn_nodonate(*dev_in, *dev_z)
            jax.block_until_ready(out)
            best = min(best, (_time.perf_counter() - t0) / inner)
        return best * 1e9

    def bench(self, in_maps, iters=8):
        """Return (est_ns_per_iter, results_of_last)."""
        import time as _time

        jax = self.jax
        dev_in = [
            jax.device_put(a, self.sharding) for a in self._concat_inputs(in_maps)
        ]
        zero_sets = [
            [jax.device_put(z, self.sharding) for z in self._zeros()]
            for _ in range(iters + 1)
        ]
        out = self.fn(*dev_in, *zero_sets[0])  # warmup
        jax.block_until_ready(out)
        t0 = _time.perf_counter()
        for i in range(iters):
            out = self.fn(*dev_in, *zero_sets[i + 1])
        jax.block_until_ready(out)
        t1 = _time.perf_counter()
        est_ns = (t1 - t0) / iters * 1e9
        results = [
            {
                name: np.asarray(out[i]).reshape(B, -1, *out[i].shape[1:])[c]
                for i, name in enumerate(self.out_names)
            }
            for c in range(B)
        ]
        return est_ns, results


_RUNNER = None
_LAST_IN_MAPS = None


def _get_runner():
    global _RUNNER
    if _RUNNER is None:
        _RUNNER = _Runner()
    return _RUNNER


def _preprocess(x, Wz, bz, Wh, bh):
    x = np.asarray(x, dtype=np.float32)
    Wz = np.asarray(Wz, dtype=np.float32)
    bz = np.asarray(bz, dtype=np.float32)
    Wh = np.asarray(Wh, dtype=np.float32)
    bh = np.asarray(bh, dtype=np.float32)

    bf = np.float16
    xT = np.ascontiguousarray(x.transpose(0, 2, 1)).astype(bf)        # (B, D, S)
    wzT = np.ascontiguousarray(Wz.transpose(0, 2, 1)).astype(bf)      # (L, D, H)
    whT = np.ascontiguousarray(Wh.transpose(0, 2, 1)).astype(bf)

    def tile_bias(b):  # (L, H) -> (L, P, HT) with [l, p, ht] = b[l, ht*P + p]
        return np.ascontiguousarray(
            b.reshape(L, HT, P).transpose(0, 2, 1)
        ).astype(np.float32)

    biases = np.ascontiguousarray(
        np.stack(
            [tile_bias(bz), tile_bias(-bz), tile_bias(bh), tile_bias(bh + 0.5)],
            axis=1,
        )
    )  # (L, 4, P, HT)

    return [
        {"xT": xT[b], "wzT": wzT, "whT": whT, "biases": biases}
        for b in range(B)
    ]


def kernel(x, Wz, bz, Wh, bh, _bench_iters=0):
    global LAST_EXEC_NS, _LAST_IN_MAPS
    runner = _get_runner()
    in_maps = _preprocess(x, Wz, bz, Wh, bh)
    _LAST_IN_MAPS = in_maps
    if _bench_iters:
        LAST_EXEC_NS, results = runner.bench(in_maps, iters=_bench_iters)
    else:
        results = runner.run(in_maps)
    out = np.stack([results[b]["outT"].T for b in range(B)], axis=0)
    return np.ascontiguousarray(out.astype(np.float32))



# revision 40
# speedup vs baseline: 62.5695x; 62.5695x over previous
"""MinGRU (L=2, B=8, S=2048, D=H=1024) Trainium2 Bass kernel.

Sharding: data-parallel over batch B across the 8 NeuronCores (1 sequence
per core); weights replicated.

Mixed-precision PE plan (measured: fp8e4 DoubleRow matmul = 216ns/instr,
same as fp16, but 2x contraction per instr -> 2x PE throughput; the
candidate (Wh) paths are precision-critical, the gate (Wz) paths are not):
  layer 0: both paths fp8e4 DoubleRow        (16 DR instr / unit)
  layer 1: gate path fp8e4 DR, cand. fp16    (8 DR + 16 fp16 / unit)
CPU-simulated rel_absmax error of this scheme: 1.26% (gate 2%).

Per-core dataflow (all shapes per core):
  inputs (host-preprocessed): x8 (D,S) fp8e4, wz0/wh0/wz1 (D,H) fp8e4,
  wh1 (D,H) fp16, biases as (2,4,128,H/128) fp32 tiles (bz,-bz,bh,bh+.5).
  unit = (layer, sb-pair, ht): psum pk/pp [128,1024] (2 banks each,
  2 sb-halves of 512 accumulated separately, weights reused across the
  pair halving LDWEIGHTS). Epilogue on [128,1024]:
    ACT: z = sigmoid(k+bz), c = sigmoid(-(k+bz)), sg = sigmoid(p+bh)
    DVE: g = (p + (bh+.5)) max sg   (exact piecewise-g identity)
    Pool: v = z*g (fp16)
    DVE: h[t] = c[t]*h[t-1] + v[t], h0=0.5 (tensor_tensor_scan, fp32 state)
  layer-0 scan emits fp16 into h1 (the layer-1 fp16 rhs); Pool converts
  h1 -> fp8 for the layer-1 gate path. layer-1 scan emits fp32 chunks
  DMA'd to DRAM as (H,S); host transposes back to (S,H).
  Linear-space scan is numerically safe: all terms positive, h in
  [~1e-3, ~4], fp32 state.
"""

import os
import sys

for _p in (
    "/root/.axon_site",
    "/root/.axon_site/_ro/trn_rl_repo",
    "/root/.axon_site/_ro/pypackages",
    "/opt/trn_rl_repo",
    "/opt/pypackages",
):
    if os.path.isdir(_p) and _p not in sys.path:
        sys.path.append(_p)

from contextlib import ExitStack

import numpy as np

import concourse.bacc as bacc
import concourse.bass as bass
import concourse.tile as tile
from concourse import mybir

L, B, S, D, H = 2, 8, 2048, 1024, 1024
P = 128
DT = D // P          # 8 contraction tiles
HT = H // P          # 8 output-channel tiles
SB = 512             # matmul free size (one PSUM bank of fp32)
SBP = 2 * SB         # epilogue unit width (2 banks)
NSBP = S // SBP      # 2 sb-pairs

F16 = mybir.dt.float16
F32 = mybir.dt.float32
F8 = mybir.dt.float8e4
AF = mybir.ActivationFunctionType
OP = mybir.AluOpType
DR = mybir.MatmulPerfMode.DoubleRow

LAST_EXEC_NS = None

N_WARM = 28


def _build(reps=1, split_last=4, n_warm=N_WARM):
    nc = bacc.Bacc("TRN2", target_bir_lowering=False, debug=False)

    x8d = nc.dram_tensor("x8", (D, S), F8, kind="ExternalInput")
    wz0d = nc.dram_tensor("wz0", (D, H), F8, kind="ExternalInput")
    wh0d = nc.dram_tensor("wh0", (D, H), F8, kind="ExternalInput")
    wz1d = nc.dram_tensor("wz1", (D, H), F8, kind="ExternalInput")
    wh1d = nc.dram_tensor("wh1", (D, H), F16, kind="ExternalInput")
    # biases pre-tiled on host: [l, f, p, ht] = bias_f[l, ht*128 + p]
    # f in (bz, -bz, bh, bh+0.5)
    bias_d = nc.dram_tensor("biases", (L, 4, P, HT), F32, kind="ExternalInput")
    outT = nc.dram_tensor("outT", (H, S), F32, kind="ExternalOutput")

    with tile.TileContext(nc) as tc, ExitStack() as ctx:
        persist = ctx.enter_context(tc.tile_pool(name="persist", bufs=1))
        zpool = ctx.enter_context(tc.tile_pool(name="zs", bufs=4))
        cvpool = ctx.enter_context(tc.tile_pool(name="cv", bufs=4))
        # layer-1 output chunks: same-ht chunks are HT units apart in
        # sbp-major order; keep enough slots live for the scan chaining
        ochunk_pool = ctx.enter_context(tc.tile_pool(name="ochunk", bufs=HT + 2))
        pk_pool = ctx.enter_context(tc.tile_pool(name="pk", bufs=2, space="PSUM"))
        pp_pool = ctx.enter_context(tc.tile_pool(name="pp", bufs=2, space="PSUM"))

        # ---- persistent SBUF state ----
        # NOTE: allocation order controls SBUF addresses. h1_sb (the layer-0
        # scan destination) is placed LAST so it does not share an SBUF bank
        # with the PE-streamed tiles (x_sb/weights): concurrent PE rhs reads
        # from an adjacent bank were measured to halve DVE scan throughput.
        x_sb = persist.tile([P, DT, S], F8)        # layer-0 input
        # h1/h18 split per sb-pair: readers of sb-pair 0 must not wait on
        # the layer-0 sb-pair-1 scans (tile-granular dependency tracking)
        h18_sb = [persist.tile([P, HT, SBP], F8, name=f"h18_{i}")
                  for i in range(NSBP)]            # fp8 copy = l1 gate rhs
        w_sb = {
            ("wz", 0): persist.tile([P, DT, H], F8, name="wz0_sb"),
            ("wh", 0): persist.tile([P, DT, H], F8, name="wh0_sb"),
            ("wz", 1): persist.tile([P, DT, H], F8, name="wz1_sb"),
            ("wh", 1): persist.tile([P, DT, H], F16, name="wh1_sb"),
        }
        h1_sb = [persist.tile([P, HT, SBP], F16, name=f"h1_{i}")
                 for i in range(NSBP)]             # layer-0 out = l1 fp16 rhs
        w_dram = {("wz", 0): wz0d, ("wh", 0): wh0d,
                  ("wz", 1): wz1d, ("wh", 1): wh1d}
        bias_tiles = [
            persist.tile([P, 4, HT], F32, name=f"bias{l}_sb") for l in range(L)
        ]
        bias_sb = {}
        for l in range(L):
            for fi, nm in enumerate(("bz", "bzn", "bh", "bh05")):
                bias_sb[(nm, l)] = bias_tiles[l][:, fi]

        def load_w(nm, l, h0, h1):
            src = w_dram[(nm, l)].rearrange("(dt p) h -> p dt h", p=P)
            nc.sync.dma_start(out=w_sb[(nm, l)][:, :, h0:h1], in_=src[:, :, h0:h1])

        x_r = x8d.rearrange("(dt p) s -> p dt s", p=P)

        def load_x(sb):
            nc.sync.dma_start(
                out=x_sb[:, :, sb * SB : (sb + 1) * SB],
                in_=x_r[:, :, sb * SB : (sb + 1) * SB],
            )

        # PE warmup: dummy matmuls on a zeroed tile run during the DMA
        # lead-in so the HAM clock gate reaches 2.4 GHz before real work.
        warm = persist.tile([P, SB], F16, name="warm")
        nc.vector.memset(warm, 0.0)
        warm_ps = pk_pool.tile([P, SBP], F32, name="pk")
        for _ in range(n_warm):
            nc.tensor.matmul(warm_ps[:, :SB], warm[:, :P], warm, start=True, stop=True)

        # DMA emission in first-consumption order (per-DMA queue overhead
        # is significant, batch where possible).
        load_w("wz", 0, 0, P)        # first unit's gate weights (0.125 MB)
        load_w("wh", 0, 0, P)
        load_x(0)
        load_x(1)
        load_w("wz", 0, P, H)
        load_w("wh", 0, P, H)
        nc.sync.dma_start(
            out=bias_tiles[0], in_=bias_d[0].rearrange("f p ht -> p f ht")
        )
        load_x(2)
        load_x(3)
        nc.sync.dma_start(
            out=bias_tiles[1], in_=bias_d[1].rearrange("f p ht -> p f ht")
        )
        for half in range(2):
            load_w("wz", 1, half * (H // 2), (half + 1) * (H // 2))
            load_w("wh", 1, half * (H // 2), (half + 1) * (H // 2))

        def mm_group_f8dr(ps, wmat, rhs8, h0, h1, s0):
            # K=1024 via 4 DoubleRow instrs per sb-half; weights reused
            # across the two halves (LDWEIGHTS once per j, 2 matmuls)
            for j in range(DT // 2):
                for sl in range(2):
                    nc.tensor.matmul(
                        ps[:, sl * SB : (sl + 1) * SB],
                        wmat[:, 2 * j : 2 * j + 2, h0:h1],
                        rhs8[:, 2 * j : 2 * j + 2, s0 + sl * SB : s0 + (sl + 1) * SB],
                        start=(j == 0),
                        stop=(j == DT // 2 - 1),
                        perf_mode=DR,
                        skip_group_check=True,
                    )

        def mm_group_f16(ps, wmat, rhs16, h0, h1, s0):
            for dt_i in range(DT):
                for sl in range(2):
                    nc.tensor.matmul(
                        ps[:, sl * SB : (sl + 1) * SB],
                        wmat[:, dt_i, h0:h1],
                        rhs16[:, dt_i, s0 + sl * SB : s0 + (sl + 1) * SB],
                        start=(dt_i == 0),
                        stop=(dt_i == DT - 1),
                        skip_group_check=True,
                    )

        def mm_group_f16_half(ps, wmat, rhs16, h0, h1, s0, sl):
            for dt_i in range(DT):
                nc.tensor.matmul(
                    ps[:, sl * SB : (sl + 1) * SB],
                    wmat[:, dt_i, h0:h1],
                    rhs16[:, dt_i, s0 + sl * SB : s0 + (sl + 1) * SB],
                    start=(dt_i == 0),
                    stop=(dt_i == DT - 1),
                    skip_group_check=True,
                )

        # deferred h1 -> fp8 casts: emitted off the critical chain, spread
        # over later units on whichever engine has slack in that phase
        cast_queue = []
        cast_rr = [0]

        def drain_casts(engines, n=1, min_queue=0):
            for _ in range(n):
                if len(cast_queue) <= min_queue:
                    return
                ht, col0, w = cast_queue.pop(0)
                sp, c0 = col0 // SBP, col0 % SBP
                dst = h18_sb[sp][:, ht, c0 : c0 + w]
                src = h1_sb[sp][:, ht, c0 : c0 + w]
                eng = engines[cast_rr[0] % len(engines)]
                cast_rr[0] += 1
                if eng == "act":
                    nc.scalar.activation(dst, src, AF.Copy)
                elif eng == "pool":
                    nc.gpsimd.tensor_copy(dst, src)
                else:
                    nc.vector.tensor_copy(dst, src)

        def emit_unit(l, sbp, ht, out_writer, drain_n=1):
            wz = w_sb[("wz", l)]
            wh = w_sb[("wh", l)]
            bz_t = bias_sb[("bz", l)]
            bzn_t = bias_sb[("bzn", l)]
            bh_t = bias_sb[("bh", l)]
            bh05_t = bias_sb[("bh05", l)]
            s0 = sbp * SBP
            if True:
                if True:
                    # casts ahead of this unit's matmuls: layer-1 gate
                    # matmuls consume h18, so finish pending chunks early.
                    # NOT on DVE: its sem orders the h18-read waits, and a
                    # lagging DVE queue stalls every sb-pair-1 unit's ldw.
                    if l == 1:
                        drain_casts(("act", "pool"), n=drain_n)
                    h0, h1 = ht * P, (ht + 1) * P
                    pk = pk_pool.tile([P, SBP], F32, name="pk")
                    pp = pp_pool.tile([P, SBP], F32, name="pp")
                    last = l == 1 and sbp == NSBP - 1 and ht >= HT - 2
                    # cand. path first: sg/g depend only on p, so they
                    # overlap the gate path's matmuls. The very last unit
                    # runs the gate path FIRST so z/c compute during the
                    # cand. matmuls (short tail).
                    if l == 0:
                        mm_group_f8dr(pp, wh, x_sb, h0, h1, s0)
                        mm_group_f8dr(pk, wz, x_sb, h0, h1, s0)
                    elif last:
                        mm_group_f8dr(pk, wz, h18_sb[sbp], h0, h1, 0)
                        mm_group_f16(pp, wh, h1_sb[sbp], h0, h1, 0)
                    else:
                        mm_group_f16(pp, wh, h1_sb[sbp], h0, h1, 0)
                        mm_group_f8dr(pk, wz, h18_sb[sbp], h0, h1, 0)
                    # fast-drain epilogue for the kernel tail: z/c full-width
                    # up front (pk group ran first), then fine chunks with
                    # everything after ACT on DVE (fewer cross-engine hops)
                    sub = split_last if last else 1
                    w = SBP // sub
                    z_full = cc_full = None
                    if last:
                        z_full = zpool.tile([P, SBP], F16, name="z")
                        cc_full = cvpool.tile([P, SBP], F16, name="cc")
                        nc.scalar.activation(
                            z_full, pk, AF.Sigmoid,
                            bias=bz_t[:, ht : ht + 1], scale=1.0,
                        )
                        nc.scalar.activation(
                            cc_full, pk, AF.Sigmoid,
                            bias=bzn_t[:, ht : ht + 1], scale=-1.0,
                        )
                    for si in range(sub):
                        c0 = si * w
                        sg = zpool.tile([P, SBP], F16, name="sg")[:, :w]
                        # ACT order matches the dependency chain: g needs sg
                        # first, the scan needs c last
                        nc.scalar.activation(
                            sg, pp[:, c0 : c0 + w], AF.Sigmoid,
                            bias=bh_t[:, ht : ht + 1], scale=1.0,
                        )
                        g = zpool.tile([P, SBP], F16, name="g")[:, :w]
                        # g = (p + (bh+0.5)) max sg. Layer 1 routes p+bh05
                        # through ACT so the DVE op is a short all-SBUF fp16
                        # max: the DVE stt was the last PSUM reader, and its
                        # latency behind the scan stalls the unit+2 matmuls
                        # (PSUM double-buffer WAR).
                        if l == 1 and not last:
                            pl = zpool.tile([P, SBP], F16, name="pl")[:, :w]
                            nc.scalar.activation(
                                pl, pp[:, c0 : c0 + w], AF.Identity,
                                bias=bh05_t[:, ht : ht + 1], scale=1.0,
                            )
                            nc.vector.tensor_max(g, pl, sg)
                        else:
                            nc.vector.scalar_tensor_tensor(
                                g, pp[:, c0 : c0 + w], bh05_t[:, ht : ht + 1],
                                sg, op0=OP.add, op1=OP.max,
                            )
                        if last:
                            z = z_full[:, c0 : c0 + w]
                            cc = cc_full[:, c0 : c0 + w]
                            v = cvpool.tile([P, SBP], F16, name="v")[:, :w]
                            nc.vector.tensor_mul(v, z, g)
                        else:
                            z = zpool.tile([P, SBP], F16, name="z")
                            cc = cvpool.tile([P, SBP], F16, name="cc")
                            v = cvpool.tile([P, SBP], F16, name="v")
                            nc.scalar.activation(
                                z, pk, AF.Sigmoid,
                                bias=bz_t[:, ht : ht + 1], scale=1.0,
                            )
                            nc.scalar.activation(
                                cc, pk, AF.Sigmoid,
                                bias=bzn_t[:, ht : ht + 1], scale=-1.0,
                            )
                            nc.gpsimd.tensor_mul(v, z, g)
                        out_writer(ht, s0 + c0, w, cc, v)
                    # during layer 0, drain casts one unit-sweep late so the
                    # cast never sits between v's on the Pool queue
                    if l == 0 and drain_n:
                        drain_casts(("act", "pool"), n=drain_n, min_queue=HT)

        # layer 0: scan into h1_sb (f16), chained over blocks; the fp8
        # convert is emitted inline right after each scan, on the engine
        # with slack in that phase (ACT during sb-pair 0's elementwise-bound
        # stretch, Pool during the PE-bound interleave) — batching casts at
        # phase boundaries gated the next phase's matmuls for ~5us each.
        def l0_writer(ht, col0, w, cc, v):
            sp, c0 = col0 // SBP, col0 % SBP
            dst = h1_sb[sp][:, ht, c0 : c0 + w]
            if col0 == 0:
                init = 0.5
            elif c0 == 0:
                init = h1_sb[sp - 1][:, ht, SBP - 1 : SBP]
            else:
                init = h1_sb[sp][:, ht, c0 - 1 : c0]
            nc.vector.tensor_tensor_scan(dst, cc, v, init, op0=OP.mult, op1=OP.add)
            dst8 = h18_sb[sp][:, ht, c0 : c0 + w]
            if sp == 0:
                nc.scalar.activation(dst8, dst, AF.Copy)
            else:
                nc.gpsimd.tensor_copy(dst8, dst)

        # layer 1: scan into fp32 chunks, DMA out per chunk
        prev_chunk = {}

        def l1_writer(ht, col0, w, cc, v):
            oc = ochunk_pool.tile([P, SBP], F32, name="oc")[:, :w]
            if col0 == 0:
                init = 0.5
            else:
                pt, pw = prev_chunk[ht]
                init = pt[:, pw - 1 : pw]
            nc.vector.tensor_tensor_scan(oc, cc, v, init, op0=OP.mult, op1=OP.add)
            prev_chunk[ht] = (oc, w)
            nc.sync.dma_start(out=outT[ht * P : (ht + 1) * P, col0 : col0 + w], in_=oc)

        def body():
            # Phase A: layer-0 sb-pair 0 alone (elementwise-bound).
            for ht in range(HT):
                emit_unit(0, 0, ht, l0_writer, drain_n=0)
            # Phase B/C: interleave layer-0 sb-pair-1 units (elementwise-
            # heavy, PE-light) with layer-1 sb-pair-0 units (PE-heavy) so
            # the PE never idles long enough to down-clock.
            seq = [(0, 1, 0), (0, 1, 1)]
            for i in range(2, HT):
                seq.append((0, 1, i))
                seq.append((1, 0, i - 2))
            seq += [(1, 0, HT - 2), (1, 0, HT - 1)]
            for l, sbp, ht in seq:
                emit_unit(l, sbp, ht, l0_writer if l == 0 else l1_writer,
                          drain_n=0)
            # Phase D: layer-1 sb-pair 1 (PE-bound)
            for ht in range(HT):
                emit_unit(1, 1, ht, l1_writer, drain_n=0)

        if reps == 1:
            body()
        else:
            with tc.For_i(0, reps, 1, hint_engines=tuple(nc.engines)):
                body()

    nc.finalize()
    return nc


class _Runner:
    """Compile the bass module once into a jitted shard_map over 8 cores."""

    def __init__(self, reps=1, **build_kwargs):
        import jax
        from jax.experimental.shard_map import shard_map
        from jax.sharding import Mesh, NamedSharding, PartitionSpec

        from concourse import bass2jax, mybir as _mybir

        self.jax = jax
        nc = _build(reps, **build_kwargs)
        self.nc = nc
        bass2jax.install_neuronx_cc_hook()

        partition_name = (
            nc.partition_id_tensor.name if nc.partition_id_tensor else None
        )
        in_names, out_names, out_avals, zero_shapes = [], [], [], []
        for alloc in nc.m.functions[0].allocations:
            if not isinstance(_mybir.MemoryLocationSet, type) or not isinstance(
                alloc, _mybir.MemoryLocationSet
            ):
                continue
            name = alloc.memorylocations[0].name
            if alloc.kind == "ExternalInput":
                if name != partition_name:
                    in_names.append(name)
            elif alloc.kind == "ExternalOutput":
                shape = tuple(alloc.tensor_shape)
                dtype = _mybir.dt.np(alloc.dtype)
                out_names.append(name)
                out_avals.append(jax.core.ShapedArray(shape, dtype))
                zero_shapes.append((shape, dtype))
        self.in_names = list(in_names)
        self.out_names = out_names
        self.zero_shapes = zero_shapes
        n_params = len(in_names)
        n_outs = len(out_names)
        all_in_names = in_names + out_names
        if partition_name is not None:
            all_in_names.append(partition_name)
        donate = tuple(range(n_params, n_params + n_outs))

        def _body(*args):
            operands = list(args)
            if partition_name is not None:
                operands.append(bass2jax.partition_id_tensor())
            outs = bass2jax._bass_exec_p.bind(
                *operands,
                out_avals=tuple(out_avals),
                in_names=tuple(all_in_names),
                out_names=tuple(out_names),
                lowering_input_output_aliases=(),
                sim_require_finite=True,
                sim_require_nnan=True,
                nc=nc,
            )
            return tuple(outs)

        self._base_body = _body
        devices = jax.devices()[:B]
        assert len(devices) == B
        self.mesh = Mesh(np.asarray(devices), ("core",))
        self.sharding = NamedSharding(self.mesh, PartitionSpec("core"))
        in_specs = (PartitionSpec("core"),) * (n_params + n_outs)
        out_specs = (PartitionSpec("core"),) * n_outs
        _mapped = shard_map(
            _body,
            mesh=self.mesh,
            in_specs=in_specs,
            out_specs=out_specs,
            check_rep=False,
        )
        self.fn = jax.jit(_mapped, donate_argnums=donate, keep_unused=True)
        self.fn_nodonate = jax.jit(_mapped, keep_unused=True)

    def _concat_inputs(self, in_maps):
        return [
            np.concatenate([np.asarray(m[name]) for m in in_maps], axis=0)
            for name in self.in_names
        ]

    def _zeros(self):
        return [
            np.zeros((B * s[0], *s[1:]), dt) for (s, dt) in self.zero_shapes
        ]

    def run(self, in_maps):
        out_arrs = self.fn(*self._concat_inputs(in_maps), *self._zeros())
        return [
            {
                name: np.asarray(out_arrs[i]).reshape(B, -1, *out_arrs[i].shape[1:])[c]
                for i, name in enumerate(self.out_names)
            }
            for c in range(B)
        ]

    def bench(self, in_maps, iters=8):
        """Return (est_ns_per_iter, results_of_last)."""
        import time as _time

        jax = self.jax
        dev_in = [
            jax.device_put(a, self.sharding) for a in self._concat_inputs(in_maps)
        ]
        zero_sets = [
            [jax.device_put(z, self.sharding) for z in self._zeros()]
            for _ in range(iters + 1)
        ]
        out = self.fn(*dev_in, *zero_sets[0])  # warmup
        jax.block_until_ready(out)
        t0 = _time.perf_counter()
        for i in range(iters):
            out = self.fn(*dev_in, *zero_sets[i + 1])
        jax.block_until_ready(out)
        t1 = _time.perf_counter()
        est_ns = (t1 - t0) / iters * 1e9
        results = [
            {
                name: np.asarray(out[i]).reshape(B, -1, *out[i].shape[1:])[c]
                for i, name in enumerate(self.out_names)
            }
            for c in range(B)
        ]
        return est_ns, results


_RUNNER = None
_LAST_IN_MAPS = None


def _get_runner():
    global _RUNNER
    if _RUNNER is None:
        _RUNNER = _Runner()
    return _RUNNER


def _preprocess(x, Wz, bz, Wh, bh):
    import ml_dtypes

    F8NP = ml_dtypes.float8_e4m3

    x = np.asarray(x, dtype=np.float32)
    Wz = np.asarray(Wz, dtype=np.float32)
    bz = np.asarray(bz, dtype=np.float32)
    Wh = np.asarray(Wh, dtype=np.float32)
    bh = np.asarray(bh, dtype=np.float32)

    x8 = np.ascontiguousarray(x.transpose(0, 2, 1)).astype(F8NP)        # (B,D,S)
    wz0 = np.ascontiguousarray(Wz[0].T).astype(F8NP)                     # (D,H)
    wh0 = np.ascontiguousarray(Wh[0].T).astype(F8NP)
    wz1 = np.ascontiguousarray(Wz[1].T).astype(F8NP)
    wh1 = np.ascontiguousarray(Wh[1].T).astype(np.float16)

    def tile_bias(b):  # (L,H) -> (L,P,HT) with [l,p,ht] = b[l, ht*P + p]
        return np.ascontiguousarray(
            b.reshape(L, HT, P).transpose(0, 2, 1)
        ).astype(np.float32)

    biases = np.ascontiguousarray(
        np.stack(
            [tile_bias(bz), tile_bias(-bz), tile_bias(bh), tile_bias(bh + 0.5)],
            axis=1,
        )
    )  # (L, 4, P, HT)

    return [
        {"x8": x8[b], "wz0": wz0, "wh0": wh0, "wz1": wz1, "wh1": wh1,
         "biases": biases}
        for b in range(B)
    ]


def kernel(x, Wz, bz, Wh, bh, _bench_iters=0):
    global LAST_EXEC_NS, _LAST_IN_MAPS
    runner = _get_runner()
    in_maps = _preprocess(x, Wz, bz, Wh, bh)
    _LAST_IN_MAPS = in_maps
    if _bench_iters:
        LAST_EXEC_NS, results = runner.bench(in_maps, iters=_bench_iters)
    else:
        results = runner.run(in_maps)
    out = np.stack([results[b]["outT"].T for b in range(B)], axis=0)
    return np.ascontiguousarray(out.astype(np.float32))


# revision 42
# speedup vs baseline: 63.2592x; 1.0110x over previous
"""MinGRU (L=2, B=8, S=2048, D=H=1024) Trainium2 Bass kernel.

Sharding: data-parallel over batch B across the 8 NeuronCores (1 sequence
per core); weights replicated.

Mixed-precision PE plan (measured: fp8e4 DoubleRow matmul = 216ns/instr,
same as fp16, but 2x contraction per instr -> 2x PE throughput; the
candidate (Wh) paths are precision-critical, the gate (Wz) paths are not):
  layer 0: both paths fp8e4 DoubleRow        (16 DR instr / unit)
  layer 1: gate path fp8e4 DR, cand. fp16    (8 DR + 16 fp16 / unit)
CPU-simulated rel_absmax error of this scheme: 1.26% (gate 2%).

Per-core dataflow (all shapes per core):
  inputs (host-preprocessed): x8 (D,S) fp8e4, wz0/wh0/wz1 (D,H) fp8e4,
  wh1 (D,H) fp16, biases as (2,4,128,H/128) fp32 tiles (bz,-bz,bh,bh+.5).
  unit = (layer, sb-pair, ht): psum pk/pp [128,1024] (2 banks each,
  2 sb-halves of 512 accumulated separately, weights reused across the
  pair halving LDWEIGHTS). Epilogue on [128,1024]:
    ACT: z = sigmoid(k+bz), c = sigmoid(-(k+bz)), sg = sigmoid(p+bh)
    DVE: g = (p + (bh+.5)) max sg   (exact piecewise-g identity)
    Pool: v = z*g (fp16)
    DVE: h[t] = c[t]*h[t-1] + v[t], h0=0.5 (tensor_tensor_scan, fp32 state)
  layer-0 scan emits fp16 into h1 (the layer-1 fp16 rhs); Pool converts
  h1 -> fp8 for the layer-1 gate path. layer-1 scan emits fp32 chunks
  DMA'd to DRAM as (H,S); host transposes back to (S,H).
  Linear-space scan is numerically safe: all terms positive, h in
  [~1e-3, ~4], fp32 state.
"""

import os
import sys

for _p in (
    "/root/.axon_site",
    "/root/.axon_site/_ro/trn_rl_repo",
    "/root/.axon_site/_ro/pypackages",
    "/opt/trn_rl_repo",
    "/opt/pypackages",
):
    if os.path.isdir(_p) and _p not in sys.path:
        sys.path.append(_p)

from contextlib import ExitStack

import numpy as np

import concourse.bacc as bacc
import concourse.bass as bass
import concourse.tile as tile
from concourse import mybir

L, B, S, D, H = 2, 8, 2048, 1024, 1024
P = 128
DT = D // P          # 8 contraction tiles
HT = H // P          # 8 output-channel tiles
SB = 512             # matmul free size (one PSUM bank of fp32)
SBP = 2 * SB         # epilogue unit width (2 banks)
NSBP = S // SBP      # 2 sb-pairs

F16 = mybir.dt.float16
F32 = mybir.dt.float32
F8 = mybir.dt.float8e4
AF = mybir.ActivationFunctionType
OP = mybir.AluOpType
DR = mybir.MatmulPerfMode.DoubleRow

LAST_EXEC_NS = None

N_WARM = 28


def _build(reps=1, split_last=4, n_warm=N_WARM):
    nc = bacc.Bacc("TRN2", target_bir_lowering=False, debug=False)

    x8d = nc.dram_tensor("x8", (D, S), F8, kind="ExternalInput")
    wz0d = nc.dram_tensor("wz0", (D, H), F8, kind="ExternalInput")
    wh0d = nc.dram_tensor("wh0", (D, H), F8, kind="ExternalInput")
    wz1d = nc.dram_tensor("wz1", (D, H), F8, kind="ExternalInput")
    wh1d = nc.dram_tensor("wh1", (D, H), F16, kind="ExternalInput")
    # biases pre-tiled on host: [l, f, p, ht] = bias_f[l, ht*128 + p]
    # f in (bz, -bz, bh, bh+0.5)
    bias_d = nc.dram_tensor("biases", (L, 4, P, HT), F32, kind="ExternalInput")
    outT = nc.dram_tensor("outT", (H, S), F32, kind="ExternalOutput")

    with tile.TileContext(nc) as tc, ExitStack() as ctx:
        persist = ctx.enter_context(tc.tile_pool(name="persist", bufs=1))
        zpool = ctx.enter_context(tc.tile_pool(name="zs", bufs=4))
        cvpool = ctx.enter_context(tc.tile_pool(name="cv", bufs=4))
        # layer-1 output chunks: same-ht chunks are HT units apart in
        # sbp-major order; keep enough slots live for the scan chaining
        ochunk_pool = ctx.enter_context(tc.tile_pool(name="ochunk", bufs=HT + 2))
        pk_pool = ctx.enter_context(tc.tile_pool(name="pk", bufs=2, space="PSUM"))
        pp_pool = ctx.enter_context(tc.tile_pool(name="pp", bufs=2, space="PSUM"))

        # ---- persistent SBUF state ----
        # NOTE: allocation order controls SBUF addresses. h1_sb (the layer-0
        # scan destination) is placed LAST so it does not share an SBUF bank
        # with the PE-streamed tiles (x_sb/weights): concurrent PE rhs reads
        # from an adjacent bank were measured to halve DVE scan throughput.
        x_sb = persist.tile([P, DT, S], F8)        # layer-0 input
        # h1/h18 split per sb-pair: readers of sb-pair 0 must not wait on
        # the layer-0 sb-pair-1 scans (tile-granular dependency tracking)
        h18_sb = [persist.tile([P, HT, SBP], F8, name=f"h18_{i}")
                  for i in range(NSBP)]            # fp8 copy = l1 gate rhs
        w_sb = {
            ("wz", 0): persist.tile([P, DT, H], F8, name="wz0_sb"),
            ("wh", 0): persist.tile([P, DT, H], F8, name="wh0_sb"),
            ("wz", 1): persist.tile([P, DT, H], F8, name="wz1_sb"),
            ("wh", 1): persist.tile([P, DT, H], F16, name="wh1_sb"),
        }
        h1_sb = [persist.tile([P, HT, SBP], F16, name=f"h1_{i}")
                 for i in range(NSBP)]             # layer-0 out = l1 fp16 rhs
        w_dram = {("wz", 0): wz0d, ("wh", 0): wh0d,
                  ("wz", 1): wz1d, ("wh", 1): wh1d}
        bias_tiles = [
            persist.tile([P, 4, HT], F32, name=f"bias{l}_sb") for l in range(L)
        ]
        bias_sb = {}
        for l in range(L):
            for fi, nm in enumerate(("bz", "bzn", "bh", "bh05")):
                bias_sb[(nm, l)] = bias_tiles[l][:, fi]

        def load_w(nm, l, h0, h1):
            src = w_dram[(nm, l)].rearrange("(dt p) h -> p dt h", p=P)
            nc.sync.dma_start(out=w_sb[(nm, l)][:, :, h0:h1], in_=src[:, :, h0:h1])

        x_r = x8d.rearrange("(dt p) s -> p dt s", p=P)

        def load_x(sb):
            nc.sync.dma_start(
                out=x_sb[:, :, sb * SB : (sb + 1) * SB],
                in_=x_r[:, :, sb * SB : (sb + 1) * SB],
            )

        # PE warmup: dummy matmuls on a zeroed tile run during the DMA
        # lead-in so the HAM clock gate reaches 2.4 GHz before real work.
        warm = persist.tile([P, SB], F16, name="warm")
        nc.vector.memset(warm, 0.0)
        warm_ps = pk_pool.tile([P, SBP], F32, name="pk")
        for _ in range(n_warm):
            nc.tensor.matmul(warm_ps[:, :SB], warm[:, :P], warm, start=True, stop=True)

        # DMA emission in first-consumption order (per-DMA queue overhead
        # is significant, batch where possible).
        load_w("wz", 0, 0, P)        # first unit's gate weights (0.125 MB)
        load_w("wh", 0, 0, P)
        load_x(0)
        load_x(1)
        load_w("wz", 0, P, H)
        load_w("wh", 0, P, H)
        nc.sync.dma_start(
            out=bias_tiles[0], in_=bias_d[0].rearrange("f p ht -> p f ht")
        )
        load_x(2)
        load_x(3)
        nc.sync.dma_start(
            out=bias_tiles[1], in_=bias_d[1].rearrange("f p ht -> p f ht")
        )
        for half in range(2):
            load_w("wz", 1, half * (H // 2), (half + 1) * (H // 2))
            load_w("wh", 1, half * (H // 2), (half + 1) * (H // 2))

        def mm_group_f8dr(ps, wmat, rhs8, h0, h1, s0):
            # K=1024 via 4 DoubleRow instrs per sb-half; weights reused
            # across the two halves (LDWEIGHTS once per j, 2 matmuls)
            for j in range(DT // 2):
                for sl in range(2):
                    nc.tensor.matmul(
                        ps[:, sl * SB : (sl + 1) * SB],
                        wmat[:, 2 * j : 2 * j + 2, h0:h1],
                        rhs8[:, 2 * j : 2 * j + 2, s0 + sl * SB : s0 + (sl + 1) * SB],
                        start=(j == 0),
                        stop=(j == DT // 2 - 1),
                        perf_mode=DR,
                        skip_group_check=True,
                    )

        def mm_group_f16(ps, wmat, rhs16, h0, h1, s0):
            for dt_i in range(DT):
                for sl in range(2):
                    nc.tensor.matmul(
                        ps[:, sl * SB : (sl + 1) * SB],
                        wmat[:, dt_i, h0:h1],
                        rhs16[:, dt_i, s0 + sl * SB : s0 + (sl + 1) * SB],
                        start=(dt_i == 0),
                        stop=(dt_i == DT - 1),
                        skip_group_check=True,
                    )

        def mm_group_f16_half(ps, wmat, rhs16, h0, h1, s0, sl):
            for dt_i in range(DT):
                nc.tensor.matmul(
                    ps[:, sl * SB : (sl + 1) * SB],
                    wmat[:, dt_i, h0:h1],
                    rhs16[:, dt_i, s0 + sl * SB : s0 + (sl + 1) * SB],
                    start=(dt_i == 0),
                    stop=(dt_i == DT - 1),
                    skip_group_check=True,
                )

        # deferred h1 -> fp8 casts: emitted off the critical chain, spread
        # over later units on whichever engine has slack in that phase
        cast_queue = []
        cast_rr = [0]

        def drain_casts(engines, n=1, min_queue=0):
            for _ in range(n):
                if len(cast_queue) <= min_queue:
                    return
                ht, col0, w = cast_queue.pop(0)
                sp, c0 = col0 // SBP, col0 % SBP
                dst = h18_sb[sp][:, ht, c0 : c0 + w]
                src = h1_sb[sp][:, ht, c0 : c0 + w]
                eng = engines[cast_rr[0] % len(engines)]
                cast_rr[0] += 1
                if eng == "act":
                    nc.scalar.activation(dst, src, AF.Copy)
                elif eng == "pool":
                    nc.gpsimd.tensor_copy(dst, src)
                else:
                    nc.vector.tensor_copy(dst, src)

        def emit_unit(l, sbp, ht, out_writer, drain_n=1):
            wz = w_sb[("wz", l)]
            wh = w_sb[("wh", l)]
            bz_t = bias_sb[("bz", l)]
            bzn_t = bias_sb[("bzn", l)]
            bh_t = bias_sb[("bh", l)]
            bh05_t = bias_sb[("bh05", l)]
            s0 = sbp * SBP
            if True:
                if True:
                    # casts ahead of this unit's matmuls: layer-1 gate
                    # matmuls consume h18, so finish pending chunks early.
                    # NOT on DVE: its sem orders the h18-read waits, and a
                    # lagging DVE queue stalls every sb-pair-1 unit's ldw.
                    if l == 1:
                        drain_casts(("act", "pool"), n=drain_n)
                    h0, h1 = ht * P, (ht + 1) * P
                    pk = pk_pool.tile([P, SBP], F32, name="pk")
                    pp = pp_pool.tile([P, SBP], F32, name="pp")
                    last = l == 1 and sbp == NSBP - 1 and ht >= HT - 2
                    # cand. path first: sg/g depend only on p, so they
                    # overlap the gate path's matmuls. The very last unit
                    # runs the gate path FIRST so z/c compute during the
                    # cand. matmuls (short tail).
                    if l == 0:
                        mm_group_f8dr(pp, wh, x_sb, h0, h1, s0)
                        mm_group_f8dr(pk, wz, x_sb, h0, h1, s0)
                    elif last:
                        mm_group_f8dr(pk, wz, h18_sb[sbp], h0, h1, 0)
                        mm_group_f16(pp, wh, h1_sb[sbp], h0, h1, 0)
                    else:
                        mm_group_f16(pp, wh, h1_sb[sbp], h0, h1, 0)
                        mm_group_f8dr(pk, wz, h18_sb[sbp], h0, h1, 0)
                    # fast-drain epilogue for the kernel tail: z/c full-width
                    # up front (pk group ran first), then fine chunks with
                    # everything after ACT on DVE (fewer cross-engine hops)
                    sub = split_last if last else 1
                    w = SBP // sub
                    z_full = cc_full = None
                    if last:
                        z_full = zpool.tile([P, SBP], F16, name="z")
                        cc_full = cvpool.tile([P, SBP], F16, name="cc")
                        nc.scalar.activation(
                            z_full, pk, AF.Sigmoid,
                            bias=bz_t[:, ht : ht + 1], scale=1.0,
                        )
                        nc.scalar.activation(
                            cc_full, pk, AF.Sigmoid,
                            bias=bzn_t[:, ht : ht + 1], scale=-1.0,
                        )
                    for si in range(sub):
                        c0 = si * w
                        sg = zpool.tile([P, SBP], F16, name="sg")[:, :w]
                        # ACT order matches the dependency chain: g needs sg
                        # first, the scan needs c last
                        nc.scalar.activation(
                            sg, pp[:, c0 : c0 + w], AF.Sigmoid,
                            bias=bh_t[:, ht : ht + 1], scale=1.0,
                        )
                        g = zpool.tile([P, SBP], F16, name="g")[:, :w]
                        # g = (p + (bh+0.5)) max sg. Layer 1 routes p+bh05
                        # through ACT so the DVE op is a short all-SBUF fp16
                        # max: the DVE stt was the last PSUM reader, and its
                        # latency behind the scan stalls the unit+2 matmuls
                        # (PSUM double-buffer WAR).
                        if l == 1 and not last:
                            pl = zpool.tile([P, SBP], F16, name="pl")[:, :w]
                            nc.scalar.activation(
                                pl, pp[:, c0 : c0 + w], AF.Identity,
                                bias=bh05_t[:, ht : ht + 1], scale=1.0,
                            )
                            nc.vector.tensor_max(g, pl, sg)
                        else:
                            nc.vector.scalar_tensor_tensor(
                                g, pp[:, c0 : c0 + w], bh05_t[:, ht : ht + 1],
                                sg, op0=OP.add, op1=OP.max,
                            )
                        if last:
                            z = z_full[:, c0 : c0 + w]
                            cc = cc_full[:, c0 : c0 + w]
                            v = cvpool.tile([P, SBP], F16, name="v")[:, :w]
                            nc.vector.tensor_mul(v, z, g)
                        else:
                            z = zpool.tile([P, SBP], F16, name="z")
                            cc = cvpool.tile([P, SBP], F16, name="cc")
                            v = cvpool.tile([P, SBP], F16, name="v")
                            nc.scalar.activation(
                                z, pk, AF.Sigmoid,
                                bias=bz_t[:, ht : ht + 1], scale=1.0,
                            )
                            nc.scalar.activation(
                                cc, pk, AF.Sigmoid,
                                bias=bzn_t[:, ht : ht + 1], scale=-1.0,
                            )
                            nc.gpsimd.tensor_mul(v, z, g)
                        out_writer(ht, s0 + c0, w, cc, v)
                    # during layer 0, drain casts one unit-sweep late so the
                    # cast never sits between v's on the Pool queue
                    if l == 0 and drain_n:
                        drain_casts(("act", "pool"), n=drain_n, min_queue=HT)

        # layer 0: scan into h1_sb (f16), chained over blocks; the fp8
        # convert is emitted inline right after each scan, on the engine
        # with slack in that phase (ACT during sb-pair 0's elementwise-bound
        # stretch, Pool during the PE-bound interleave) — batching casts at
        # phase boundaries gated the next phase's matmuls for ~5us each.
        def l0_writer(ht, col0, w, cc, v):
            sp, c0 = col0 // SBP, col0 % SBP
            dst = h1_sb[sp][:, ht, c0 : c0 + w]
            if col0 == 0:
                init = 0.5
            elif c0 == 0:
                init = h1_sb[sp - 1][:, ht, SBP - 1 : SBP]
            else:
                init = h1_sb[sp][:, ht, c0 - 1 : c0]
            nc.vector.tensor_tensor_scan(dst, cc, v, init, op0=OP.mult, op1=OP.add)
            dst8 = h18_sb[sp][:, ht, c0 : c0 + w]
            if sp == 0:
                nc.scalar.activation(dst8, dst, AF.Copy)
            else:
                nc.gpsimd.tensor_copy(dst8, dst)

        # layer 1: scan into fp32 chunks, DMA out per chunk
        prev_chunk = {}

        def l1_writer(ht, col0, w, cc, v):
            oc = ochunk_pool.tile([P, SBP], F32, name="oc")[:, :w]
            if col0 == 0:
                init = 0.5
            else:
                pt, pw = prev_chunk[ht]
                init = pt[:, pw - 1 : pw]
            nc.vector.tensor_tensor_scan(oc, cc, v, init, op0=OP.mult, op1=OP.add)
            prev_chunk[ht] = (oc, w)
            nc.sync.dma_start(out=outT[ht * P : (ht + 1) * P, col0 : col0 + w], in_=oc)

        def body():
            # Phase A: layer-0 sb-pair 0 alone (elementwise-bound).
            for ht in range(HT):
                emit_unit(0, 0, ht, l0_writer, drain_n=0)
            # Phase B/C: interleave layer-0 sb-pair-1 units (elementwise-
            # heavy, PE-light) with layer-1 sb-pair-0 units (PE-heavy) so
            # the PE never idles long enough to down-clock.
            seq = [(0, 1, 0), (0, 1, 1)]
            for i in range(2, HT):
                seq.append((0, 1, i))
                seq.append((1, 0, i - 2))
            seq += [(1, 0, HT - 2), (1, 0, HT - 1)]
            for l, sbp, ht in seq:
                emit_unit(l, sbp, ht, l0_writer if l == 0 else l1_writer,
                          drain_n=0)
            # Phase D: layer-1 sb-pair 1 (PE-bound)
            for ht in range(HT):
                emit_unit(1, 1, ht, l1_writer, drain_n=0)

        if reps == 1:
            body()
        else:
            with tc.For_i(0, reps, 1, hint_engines=tuple(nc.engines)):
                body()

    nc.finalize()
    return nc


class _Runner:
    """Compile the bass module once into a jitted shard_map over 8 cores."""

    def __init__(self, reps=1, **build_kwargs):
        import jax
        from jax.experimental.shard_map import shard_map
        from jax.sharding import Mesh, NamedSharding, PartitionSpec

        from concourse import bass2jax, mybir as _mybir

        self.jax = jax
        nc = _build(reps, **build_kwargs)
        self.nc = nc
        bass2jax.install_neuronx_cc_hook()

        partition_name = (
            nc.partition_id_tensor.name if nc.partition_id_tensor else None
        )
        in_names, out_names, out_avals, zero_shapes = [], [], [], []
        for alloc in nc.m.functions[0].allocations:
            if not isinstance(_mybir.MemoryLocationSet, type) or not isinstance(
                alloc, _mybir.MemoryLocationSet
            ):
                continue
            name = alloc.memorylocations[0].name
            if alloc.kind == "ExternalInput":
                if name != partition_name:
                    in_names.append(name)
            elif alloc.kind == "ExternalOutput":
                shape = tuple(alloc.tensor_shape)
                dtype = _mybir.dt.np(alloc.dtype)
                out_names.append(name)
                out_avals.append(jax.core.ShapedArray(shape, dtype))
                zero_shapes.append((shape, dtype))
        self.in_names = list(in_names)
        self.out_names = out_names
        self.zero_shapes = zero_shapes
        n_params = len(in_names)
        n_outs = len(out_names)
        all_in_names = in_names + out_names
        if partition_name is not None:
            all_in_names.append(partition_name)
        donate = tuple(range(n_params, n_params + n_outs))

        def _body(*args):
            operands = list(args)
            if partition_name is not None:
                operands.append(bass2jax.partition_id_tensor())
            outs = bass2jax._bass_exec_p.bind(
                *operands,
                out_avals=tuple(out_avals),
                in_names=tuple(all_in_names),
                out_names=tuple(out_names),
                lowering_input_output_aliases=(),
                sim_require_finite=True,
                sim_require_nnan=True,
                nc=nc,
            )
            return tuple(outs)

        self._base_body = _body
        devices = jax.devices()[:B]
        assert len(devices) == B
        self.mesh = Mesh(np.asarray(devices), ("core",))
        self.sharding = NamedSharding(self.mesh, PartitionSpec("core"))
        in_specs = (PartitionSpec("core"),) * (n_params + n_outs)
        out_specs = (PartitionSpec("core"),) * n_outs
        _mapped = shard_map(
            _body,
            mesh=self.mesh,
            in_specs=in_specs,
            out_specs=out_specs,
            check_rep=False,
        )
        self.fn = jax.jit(_mapped, donate_argnums=donate, keep_unused=True)
        self.fn_nodonate = jax.jit(_mapped, keep_unused=True)

    def _concat_inputs(self, in_maps):
        return [
            np.concatenate([np.asarray(m[name]) for m in in_maps], axis=0)
            for name in self.in_names
        ]

    def _zeros(self):
        return [
            np.zeros((B * s[0], *s[1:]), dt) for (s, dt) in self.zero_shapes
        ]

    def run(self, in_maps):
        out_arrs = self.fn(*self._concat_inputs(in_maps), *self._zeros())
        return [
            {
                name: np.asarray(out_arrs[i]).reshape(B, -1, *out_arrs[i].shape[1:])[c]
                for i, name in enumerate(self.out_names)
            }
            for c in range(B)
        ]

    def bench(self, in_maps, iters=8):
        """Return (est_ns_per_iter, results_of_last)."""
        import time as _time

        jax = self.jax
        dev_in = [
            jax.device_put(a, self.sharding) for a in self._concat_inputs(in_maps)
        ]
        zero_sets = [
            [jax.device_put(z, self.sharding) for z in self._zeros()]
            for _ in range(iters + 1)
        ]
        out = self.fn(*dev_in, *zero_sets[0])  # warmup
        jax.block_until_ready(out)
        t0 = _time.perf_counter()
        for i in range(iters):
            out = self.fn(*dev_in, *zero_sets[i + 1])
        jax.block_until_ready(out)
        t1 = _time.perf_counter()
        est_ns = (t1 - t0) / iters * 1e9
        results = [
            {
                name: np.asarray(out[i]).reshape(B, -1, *out[i].shape[1:])[c]
                for i, name in enumerate(self.out_names)
            }
            for c in range(B)
        ]
        return est_ns, results


_RUNNER = None
_LAST_IN_MAPS = None


def _get_runner():
    global _RUNNER
    if _RUNNER is None:
        _RUNNER = _Runner()
    return _RUNNER


def _preprocess(x, Wz, bz, Wh, bh):
    import ml_dtypes

    F8NP = ml_dtypes.float8_e4m3

    x = np.asarray(x, dtype=np.float32)
    Wz = np.asarray(Wz, dtype=np.float32)
    bz = np.asarray(bz, dtype=np.float32)
    Wh = np.asarray(Wh, dtype=np.float32)
    bh = np.asarray(bh, dtype=np.float32)

    x8 = np.ascontiguousarray(x.transpose(0, 2, 1)).astype(F8NP)        # (B,D,S)
    wz0 = np.ascontiguousarray(Wz[0].T).astype(F8NP)                     # (D,H)
    wh0 = np.ascontiguousarray(Wh[0].T).astype(F8NP)
    wz1 = np.ascontiguousarray(Wz[1].T).astype(F8NP)
    wh1 = np.ascontiguousarray(Wh[1].T).astype(np.float16)

    def tile_bias(b):  # (L,H) -> (L,P,HT) with [l,p,ht] = b[l, ht*P + p]
        return np.ascontiguousarray(
            b.reshape(L, HT, P).transpose(0, 2, 1)
        ).astype(np.float32)

    biases = np.ascontiguousarray(
        np.stack(
            [tile_bias(bz), tile_bias(-bz), tile_bias(bh), tile_bias(bh + 0.5)],
            axis=1,
        )
    )  # (L, 4, P, HT)

    return [
        {"x8": x8[b], "wz0": wz0, "wh0": wh0, "wz1": wz1, "wh1": wh1,
         "biases": biases}
        for b in range(B)
    ]


def kernel(x, Wz, bz, Wh, bh, _bench_iters=0):
    global LAST_EXEC_NS, _LAST_IN_MAPS
    runner = _get_runner()
    in_maps = _preprocess(x, Wz, bz, Wh, bh)
    _LAST_IN_MAPS = in_maps
    if _bench_iters:
        LAST_EXEC_NS, results = runner.bench(in_maps, iters=_bench_iters)
    else:
        results = runner.run(in_maps)
    out = np.stack([results[b]["outT"].T for b in range(B)], axis=0)
    return np.ascontiguousarray(out.astype(np.float32))


# revision 47
# speedup vs baseline: 69.5525x; 1.0995x over previous
"""MinGRU (L=2, B=8, S=2048, D=H=1024) Trainium2 Bass kernel.

Sharding: data-parallel over batch B across the 8 NeuronCores (1 sequence
per core); weights replicated.

Mixed-precision PE plan (measured: fp8e4 DoubleRow matmul = 216ns/instr,
same as fp16, but 2x contraction per instr -> 2x PE throughput; the
candidate (Wh) paths are precision-critical, the gate (Wz) paths are not):
  layer 0: both paths fp8e4 DoubleRow        (16 DR instr / unit)
  layer 1: gate path fp8e4 DR, cand. fp16    (8 DR + 16 fp16 / unit)
CPU-simulated rel_absmax error of this scheme: 1.26% (gate 2%).

Per-core dataflow (all shapes per core):
  inputs (host-preprocessed): x8 (D,S) fp8e4, wz0/wh0/wz1 (D,H) fp8e4,
  wh1 (D,H) fp16, biases as (2,4,128,H/128) fp32 tiles (bz,-bz,bh,bh+.5).
  unit = (layer, sb-pair, ht): psum pk/pp [128,1024] (2 banks each,
  2 sb-halves of 512 accumulated separately, weights reused across the
  pair halving LDWEIGHTS). Epilogue on [128,1024]:
    ACT: z = sigmoid(k+bz), c = sigmoid(-(k+bz)), sg = sigmoid(p+bh)
    DVE: g = (p + (bh+.5)) max sg   (exact piecewise-g identity)
    Pool: v = z*g (fp16)
    DVE: h[t] = c[t]*h[t-1] + v[t], h0=0.5 (tensor_tensor_scan, fp32 state)
  layer-0 scan emits fp16 into h1 (the layer-1 fp16 rhs); an inline
  copy (ACT in the elementwise-bound phase, Pool in the PE-bound
  interleave) converts h1 -> fp8 for the layer-1 gate path. layer-1
  scan emits fp32 chunks DMA'd to DRAM as (H,S); host transposes back.
  Linear-space scan is numerically safe: all terms positive, h in
  [~1e-3, ~4], fp32 state.

Schedule: A) layer-0 sb-pair 0 alone; B/C) layer-0 sb-pair 1 interleaved
with layer-1 sb-pair 0 (PE-heavy fills the elementwise-bound idle, so
the PE p-state never down-clocks); D) layer-1 sb-pair 1. h1/h18 are
split per sb-pair (tile-granular dep tracking would otherwise stall
layer-1 reads on all layer-0 scans). Measured on HW: 210.5 us vs the
319.6 us fp16 baseline; rel_absmax err 1.284% (gate 2e-2).
"""

import os
import sys

for _p in (
    "/root/.axon_site",
    "/root/.axon_site/_ro/trn_rl_repo",
    "/root/.axon_site/_ro/pypackages",
    "/opt/trn_rl_repo",
    "/opt/pypackages",
):
    if os.path.isdir(_p) and _p not in sys.path:
        sys.path.append(_p)

from contextlib import ExitStack

import numpy as np

import concourse.bacc as bacc
import concourse.bass as bass
import concourse.tile as tile
from concourse import mybir

L, B, S, D, H = 2, 8, 2048, 1024, 1024
P = 128
DT = D // P          # 8 contraction tiles
HT = H // P          # 8 output-channel tiles
SB = 512             # matmul free size (one PSUM bank of fp32)
SBP = 2 * SB         # epilogue unit width (2 banks)
NSBP = S // SBP      # 2 sb-pairs

F16 = mybir.dt.float16
F32 = mybir.dt.float32
F8 = mybir.dt.float8e4
AF = mybir.ActivationFunctionType
OP = mybir.AluOpType
DR = mybir.MatmulPerfMode.DoubleRow

LAST_EXEC_NS = None

N_WARM = 28


def _build(reps=1, split_last=4, n_warm=N_WARM):
    nc = bacc.Bacc("TRN2", target_bir_lowering=False, debug=False)

    x8d = nc.dram_tensor("x8", (D, S), F8, kind="ExternalInput")
    wz0d = nc.dram_tensor("wz0", (D, H), F8, kind="ExternalInput")
    wh0d = nc.dram_tensor("wh0", (D, H), F8, kind="ExternalInput")
    wz1d = nc.dram_tensor("wz1", (D, H), F8, kind="ExternalInput")
    wh1d = nc.dram_tensor("wh1", (D, H), F16, kind="ExternalInput")
    # biases pre-tiled on host: [l, f, p, ht] = bias_f[l, ht*128 + p]
    # f in (bz, -bz, bh, bh+0.5)
    bias_d = nc.dram_tensor("biases", (L, 4, P, HT), F32, kind="ExternalInput")
    outT = nc.dram_tensor("outT", (H, S), F32, kind="ExternalOutput")

    with tile.TileContext(nc) as tc, ExitStack() as ctx:
        persist = ctx.enter_context(tc.tile_pool(name="persist", bufs=1))
        zpool = ctx.enter_context(tc.tile_pool(name="zs", bufs=4))
        cvpool = ctx.enter_context(tc.tile_pool(name="cv", bufs=4))
        # layer-1 output chunks: same-ht chunks are HT units apart in
        # sbp-major order; keep enough slots live for the scan chaining
        ochunk_pool = ctx.enter_context(tc.tile_pool(name="ochunk", bufs=HT + 2))
        pk_pool = ctx.enter_context(tc.tile_pool(name="pk", bufs=2, space="PSUM"))
        pp_pool = ctx.enter_context(tc.tile_pool(name="pp", bufs=2, space="PSUM"))

        # ---- persistent SBUF state ----
        # NOTE: allocation order controls SBUF addresses. h1_sb (the layer-0
        # scan destination) is placed LAST so it does not share an SBUF bank
        # with the PE-streamed tiles (x_sb/weights): concurrent PE rhs reads
        # from an adjacent bank were measured to halve DVE scan throughput.
        x_sb = persist.tile([P, DT, S], F8)        # layer-0 input
        # h1/h18 split per sb-pair: readers of sb-pair 0 must not wait on
        # the layer-0 sb-pair-1 scans (tile-granular dependency tracking)
        h18_sb = [persist.tile([P, HT, SBP], F8, name="h18_0")]  # fp8 l1 gate rhs (sb-pair 0 only)
        w_sb = {
            ("wz", 0): persist.tile([P, DT, H], F8, name="wz0_sb"),
            ("wh", 0): persist.tile([P, DT, H], F8, name="wh0_sb"),
            ("wz", 1): persist.tile([P, DT, H], F8, name="wz1_sb"),
            ("wh", 1): persist.tile([P, DT, H], F16, name="wh1_sb"),
        }
        h1_sb = [persist.tile([P, HT, SBP], F16, name=f"h1_{i}")
                 for i in range(NSBP)]             # layer-0 out = l1 fp16 rhs
        w_dram = {("wz", 0): wz0d, ("wh", 0): wh0d,
                  ("wz", 1): wz1d, ("wh", 1): wh1d}
        bias_tiles = [
            persist.tile([P, 4, HT], F32, name=f"bias{l}_sb") for l in range(L)
        ]
        bias_sb = {}
        for l in range(L):
            for fi, nm in enumerate(("bz", "bzn", "bh", "bh05")):
                bias_sb[(nm, l)] = bias_tiles[l][:, fi]

        def load_w(nm, l, h0, h1):
            src = w_dram[(nm, l)].rearrange("(dt p) h -> p dt h", p=P)
            nc.sync.dma_start(out=w_sb[(nm, l)][:, :, h0:h1], in_=src[:, :, h0:h1])

        x_r = x8d.rearrange("(dt p) s -> p dt s", p=P)

        def load_x(sb):
            nc.sync.dma_start(
                out=x_sb[:, :, sb * SB : (sb + 1) * SB],
                in_=x_r[:, :, sb * SB : (sb + 1) * SB],
            )

        # PE warmup: dummy matmuls on a zeroed tile run during the DMA
        # lead-in so the HAM clock gate reaches 2.4 GHz before real work.
        warm = persist.tile([P, SB], F16, name="warm")
        nc.vector.memset(warm, 0.0)
        warm_ps = pk_pool.tile([P, SBP], F32, name="pk")
        for _ in range(n_warm):
            nc.tensor.matmul(warm_ps[:, :SB], warm[:, :P], warm, start=True, stop=True)

        # DMA emission in first-consumption order (per-DMA queue overhead
        # is significant, batch where possible).
        load_w("wz", 0, 0, P)        # first unit's gate weights (0.125 MB)
        load_w("wh", 0, 0, P)
        load_x(0)
        load_x(1)
        load_w("wz", 0, P, H)
        load_w("wh", 0, P, H)
        nc.sync.dma_start(
            out=bias_tiles[0], in_=bias_d[0].rearrange("f p ht -> p f ht")
        )
        load_x(2)
        load_x(3)
        nc.sync.dma_start(
            out=bias_tiles[1], in_=bias_d[1].rearrange("f p ht -> p f ht")
        )
        for half in range(2):
            load_w("wz", 1, half * (H // 2), (half + 1) * (H // 2))
            load_w("wh", 1, half * (H // 2), (half + 1) * (H // 2))

        def mm_group_f8dr(ps, wmat, rhs8, h0, h1, s0):
            # K=1024 via 4 DoubleRow instrs per sb-half; weights reused
            # across the two halves (LDWEIGHTS once per j, 2 matmuls)
            for j in range(DT // 2):
                for sl in range(2):
                    nc.tensor.matmul(
                        ps[:, sl * SB : (sl + 1) * SB],
                        wmat[:, 2 * j : 2 * j + 2, h0:h1],
                        rhs8[:, 2 * j : 2 * j + 2, s0 + sl * SB : s0 + (sl + 1) * SB],
                        start=(j == 0),
                        stop=(j == DT // 2 - 1),
                        perf_mode=DR,
                        skip_group_check=True,
                    )

        def mm_group_f16(ps, wmat, rhs16, h0, h1, s0):
            for dt_i in range(DT):
                for sl in range(2):
                    nc.tensor.matmul(
                        ps[:, sl * SB : (sl + 1) * SB],
                        wmat[:, dt_i, h0:h1],
                        rhs16[:, dt_i, s0 + sl * SB : s0 + (sl + 1) * SB],
                        start=(dt_i == 0),
                        stop=(dt_i == DT - 1),
                        skip_group_check=True,
                    )

        def mm_group_f16_half(ps, wmat, rhs16, h0, h1, s0, sl):
            for dt_i in range(DT):
                nc.tensor.matmul(
                    ps[:, sl * SB : (sl + 1) * SB],
                    wmat[:, dt_i, h0:h1],
                    rhs16[:, dt_i, s0 + sl * SB : s0 + (sl + 1) * SB],
                    start=(dt_i == 0),
                    stop=(dt_i == DT - 1),
                    skip_group_check=True,
                )

        # deferred h1 -> fp8 casts: emitted off the critical chain, spread
        # over later units on whichever engine has slack in that phase
        cast_queue = []
        cast_rr = [0]

        def drain_casts(engines, n=1, min_queue=0):
            for _ in range(n):
                if len(cast_queue) <= min_queue:
                    return
                ht, col0, w = cast_queue.pop(0)
                sp, c0 = col0 // SBP, col0 % SBP
                dst = h18_sb[sp][:, ht, c0 : c0 + w]
                src = h1_sb[sp][:, ht, c0 : c0 + w]
                eng = engines[cast_rr[0] % len(engines)]
                cast_rr[0] += 1
                if eng == "act":
                    nc.scalar.activation(dst, src, AF.Copy)
                elif eng == "pool":
                    nc.gpsimd.tensor_copy(dst, src)
                else:
                    nc.vector.tensor_copy(dst, src)

        def emit_unit(l, sbp, ht, out_writer, drain_n=1):
            wz = w_sb[("wz", l)]
            wh = w_sb[("wh", l)]
            bz_t = bias_sb[("bz", l)]
            bzn_t = bias_sb[("bzn", l)]
            bh_t = bias_sb[("bh", l)]
            bh05_t = bias_sb[("bh05", l)]
            s0 = sbp * SBP
            if True:
                if True:
                    # casts ahead of this unit's matmuls: layer-1 gate
                    # matmuls consume h18, so finish pending chunks early.
                    # NOT on DVE: its sem orders the h18-read waits, and a
                    # lagging DVE queue stalls every sb-pair-1 unit's ldw.
                    if l == 1:
                        drain_casts(("act", "pool"), n=drain_n)
                    h0, h1 = ht * P, (ht + 1) * P
                    pk = pk_pool.tile([P, SBP], F32, name="pk")
                    pp = pp_pool.tile([P, SBP], F32, name="pp")
                    last = l == 1 and sbp == NSBP - 1 and ht >= HT - 3
                    # cand. path first: sg/g depend only on p, so they
                    # overlap the gate path's matmuls. The very last unit
                    # runs the gate path FIRST so z/c compute during the
                    # cand. matmuls (short tail).
                    # layer-1 sb-pair 1 reads the gate rhs in fp16 (mixed
                    # fp8-weights x fp16-rhs plain matmuls): the fp8 casts
                    # for that half ran on a saturated Pool queue and their
                    # backlog cascaded into a ~30us post-matmul tail.
                    if l == 0:
                        mm_group_f8dr(pp, wh, x_sb, h0, h1, s0)
                        mm_group_f8dr(pk, wz, x_sb, h0, h1, s0)
                    elif last:
                        mm_group_f16(pk, wz, h1_sb[sbp], h0, h1, 0)
                        mm_group_f16(pp, wh, h1_sb[sbp], h0, h1, 0)
                    elif sbp == NSBP - 1:
                        mm_group_f16(pp, wh, h1_sb[sbp], h0, h1, 0)
                        mm_group_f16(pk, wz, h1_sb[sbp], h0, h1, 0)
                    else:
                        mm_group_f16(pp, wh, h1_sb[sbp], h0, h1, 0)
                        mm_group_f8dr(pk, wz, h18_sb[sbp], h0, h1, 0)
                    # fast-drain epilogue for the kernel tail: z/c full-width
                    # up front (pk group ran first), then fine chunks with
                    # everything after ACT on DVE (fewer cross-engine hops)
                    sub = split_last if last else 1
                    w = SBP // sub
                    z_full = cc_full = None
                    if last:
                        z_full = zpool.tile([P, SBP], F16, name="z")
                        cc_full = cvpool.tile([P, SBP], F16, name="cc")
                        nc.scalar.activation(
                            z_full, pk, AF.Sigmoid,
                            bias=bz_t[:, ht : ht + 1], scale=1.0,
                        )
                        nc.scalar.activation(
                            cc_full, pk, AF.Sigmoid,
                            bias=bzn_t[:, ht : ht + 1], scale=-1.0,
                        )
                    for si in range(sub):
                        c0 = si * w
                        sg = zpool.tile([P, SBP], F16, name="sg")[:, :w]
                        # ACT order matches the dependency chain: g needs sg
                        # first, the scan needs c last
                        nc.scalar.activation(
                            sg, pp[:, c0 : c0 + w], AF.Sigmoid,
                            bias=bh_t[:, ht : ht + 1], scale=1.0,
                        )
                        g = zpool.tile([P, SBP], F16, name="g")[:, :w]
                        # g = (p + (bh+0.5)) max sg. Layer 1 routes p+bh05
                        # through ACT so the DVE op is a short all-SBUF fp16
                        # max: the DVE stt was the last PSUM reader, and its
                        # latency behind the scan stalls the unit+2 matmuls
                        # (PSUM double-buffer WAR).
                        if l == 1 and not last:
                            pl = zpool.tile([P, SBP], F16, name="pl")[:, :w]
                            nc.scalar.activation(
                                pl, pp[:, c0 : c0 + w], AF.Identity,
                                bias=bh05_t[:, ht : ht + 1], scale=1.0,
                            )
                            nc.vector.tensor_max(g, pl, sg)
                        else:
                            nc.vector.scalar_tensor_tensor(
                                g, pp[:, c0 : c0 + w], bh05_t[:, ht : ht + 1],
                                sg, op0=OP.add, op1=OP.max,
                            )
                        if last:
                            z = z_full[:, c0 : c0 + w]
                            cc = cc_full[:, c0 : c0 + w]
                            v = cvpool.tile([P, SBP], F16, name="v")[:, :w]
                            nc.vector.tensor_mul(v, z, g)
                        else:
                            z = zpool.tile([P, SBP], F16, name="z")
                            cc = cvpool.tile([P, SBP], F16, name="cc")
                            v = cvpool.tile([P, SBP], F16, name="v")
                            nc.scalar.activation(
                                z, pk, AF.Sigmoid,
                                bias=bz_t[:, ht : ht + 1], scale=1.0,
                            )
                            nc.scalar.activation(
                                cc, pk, AF.Sigmoid,
                                bias=bzn_t[:, ht : ht + 1], scale=-1.0,
                            )
                            nc.gpsimd.tensor_mul(v, z, g)
                        out_writer(ht, s0 + c0, w, cc, v)
                    # during layer 0, drain casts one unit-sweep late so the
                    # cast never sits between v's on the Pool queue
                    if l == 0 and drain_n:
                        drain_casts(("act", "pool"), n=drain_n, min_queue=HT)

        # layer 0: scan into h1_sb (f16), chained over blocks; the fp8
        # convert is emitted inline right after each scan, on the engine
        # with slack in that phase (ACT during sb-pair 0's elementwise-bound
        # stretch, Pool during the PE-bound interleave) — batching casts at
        # phase boundaries gated the next phase's matmuls for ~5us each.
        def l0_writer(ht, col0, w, cc, v):
            sp, c0 = col0 // SBP, col0 % SBP
            dst = h1_sb[sp][:, ht, c0 : c0 + w]
            if col0 == 0:
                init = 0.5
            elif c0 == 0:
                init = h1_sb[sp - 1][:, ht, SBP - 1 : SBP]
            else:
                init = h1_sb[sp][:, ht, c0 - 1 : c0]
            nc.vector.tensor_tensor_scan(dst, cc, v, init, op0=OP.mult, op1=OP.add)
            if sp == 0:
                # only sb-pair 0 needs the fp8 copy (layer-1 DR gate path);
                # sb-pair 1's gate matmuls read h1 fp16 directly
                dst8 = h18_sb[0][:, ht, c0 : c0 + w]
                nc.scalar.activation(dst8, dst, AF.Copy)

        # layer 1: scan into fp32 chunks, DMA out per chunk
        prev_chunk = {}

        def l1_writer(ht, col0, w, cc, v):
            oc = ochunk_pool.tile([P, SBP], F32, name="oc")[:, :w]
            if col0 == 0:
                init = 0.5
            else:
                pt, pw = prev_chunk[ht]
                init = pt[:, pw - 1 : pw]
            nc.vector.tensor_tensor_scan(oc, cc, v, init, op0=OP.mult, op1=OP.add)
            prev_chunk[ht] = (oc, w)
            nc.sync.dma_start(out=outT[ht * P : (ht + 1) * P, col0 : col0 + w], in_=oc)

        def body():
            # Phase A: layer-0 sb-pair 0 alone (elementwise-bound).
            for ht in range(HT):
                emit_unit(0, 0, ht, l0_writer, drain_n=0)
            # Phase B/C: interleave layer-0 sb-pair-1 units (elementwise-
            # heavy, PE-light) with layer-1 sb-pair-0 units (PE-heavy) so
            # the PE never idles long enough to down-clock.
            seq = [(0, 1, 0), (0, 1, 1)]
            for i in range(2, HT):
                seq.append((0, 1, i))
                seq.append((1, 0, i - 2))
            seq += [(1, 0, HT - 2), (1, 0, HT - 1)]
            for l, sbp, ht in seq:
                emit_unit(l, sbp, ht, l0_writer if l == 0 else l1_writer,
                          drain_n=0)
            # Phase D: layer-1 sb-pair 1 (PE-bound)
            for ht in range(HT):
                emit_unit(1, 1, ht, l1_writer, drain_n=0)

        if reps == 1:
            body()
        else:
            with tc.For_i(0, reps, 1, hint_engines=tuple(nc.engines)):
                body()

    nc.finalize()
    return nc


class _Runner:
    """Compile the bass module once into a jitted shard_map over 8 cores."""

    def __init__(self, reps=1, **build_kwargs):
        import jax
        from jax.experimental.shard_map import shard_map
        from jax.sharding import Mesh, NamedSharding, PartitionSpec

        from concourse import bass2jax, mybir as _mybir

        self.jax = jax
        nc = _build(reps, **build_kwargs)
        self.nc = nc
        bass2jax.install_neuronx_cc_hook()

        partition_name = (
            nc.partition_id_tensor.name if nc.partition_id_tensor else None
        )
        in_names, out_names, out_avals, zero_shapes = [], [], [], []
        for alloc in nc.m.functions[0].allocations:
            if not isinstance(_mybir.MemoryLocationSet, type) or not isinstance(
                alloc, _mybir.MemoryLocationSet
            ):
                continue
            name = alloc.memorylocations[0].name
            if alloc.kind == "ExternalInput":
                if name != partition_name:
                    in_names.append(name)
            elif alloc.kind == "ExternalOutput":
                shape = tuple(alloc.tensor_shape)
                dtype = _mybir.dt.np(alloc.dtype)
                out_names.append(name)
                out_avals.append(jax.core.ShapedArray(shape, dtype))
                zero_shapes.append((shape, dtype))
        self.in_names = list(in_names)
        self.out_names = out_names
        self.zero_shapes = zero_shapes
        n_params = len(in_names)
        n_outs = len(out_names)
        all_in_names = in_names + out_names
        if partition_name is not None:
            all_in_names.append(partition_name)
        donate = tuple(range(n_params, n_params + n_outs))

        def _body(*args):
            operands = list(args)
            if partition_name is not None:
                operands.append(bass2jax.partition_id_tensor())
            outs = bass2jax._bass_exec_p.bind(
                *operands,
                out_avals=tuple(out_avals),
                in_names=tuple(all_in_names),
                out_names=tuple(out_names),
                lowering_input_output_aliases=(),
                sim_require_finite=True,
                sim_require_nnan=True,
                nc=nc,
            )
            return tuple(outs)

        self._base_body = _body
        devices = jax.devices()[:B]
        assert len(devices) == B
        self.mesh = Mesh(np.asarray(devices), ("core",))
        self.sharding = NamedSharding(self.mesh, PartitionSpec("core"))
        in_specs = (PartitionSpec("core"),) * (n_params + n_outs)
        out_specs = (PartitionSpec("core"),) * n_outs
        _mapped = shard_map(
            _body,
            mesh=self.mesh,
            in_specs=in_specs,
            out_specs=out_specs,
            check_rep=False,
        )
        self.fn = jax.jit(_mapped, donate_argnums=donate, keep_unused=True)
        self.fn_nodonate = jax.jit(_mapped, keep_unused=True)

    def _concat_inputs(self, in_maps):
        return [
            np.concatenate([np.asarray(m[name]) for m in in_maps], axis=0)
            for name in self.in_names
        ]

    def _zeros(self):
        return [
            np.zeros((B * s[0], *s[1:]), dt) for (s, dt) in self.zero_shapes
        ]

    def run(self, in_maps):
        out_arrs = self.fn(*self._concat_inputs(in_maps), *self._zeros())
        return [
            {
                name: np.asarray(out_arrs[i]).reshape(B, -1, *out_arrs[i].shape[1:])[c]
                for i, name in enumerate(self.out_names)
            }
            for c in range(B)
        ]

    def bench(self, in_maps, iters=8):
        """Return (est_ns_per_iter, results_of_last)."""
        import time as _time

        jax = self.jax
        dev_in = [
            jax.device_put(a, self.sharding) for a in self._concat_inputs(in_maps)
        ]
        zero_sets = [
            [jax.device_put(z, self.sharding) for z in self._zeros()]
            for _ in range(iters + 1)
        ]
        out = self.fn(*dev_in, *zero_sets[0])  # warmup
        jax.block_until_ready(out)
        t0 = _time.perf_counter()
        for i in range(iters):
            out = self.fn(*dev_in, *zero_sets[i + 1])
        jax.block_until_ready(out)
        t1 = _time.perf_counter()
        est_ns = (t1 - t0) / iters * 1e9
        results = [
            {
                name: np.asarray(out[i]).reshape(B, -1, *out[i].shape[1:])[c]
                for i, name in enumerate(self.out_names)
            }
            for c in range(B)
        ]
        return est_ns, results


_RUNNER = None
_LAST_IN_MAPS = None


def _get_runner():
    global _RUNNER
    if _RUNNER is None:
        _RUNNER = _Runner()
    return _RUNNER


def _preprocess(x, Wz, bz, Wh, bh):
    import ml_dtypes

    F8NP = ml_dtypes.float8_e4m3

    x = np.asarray(x, dtype=np.float32)
    Wz = np.asarray(Wz, dtype=np.float32)
    bz = np.asarray(bz, dtype=np.float32)
    Wh = np.asarray(Wh, dtype=np.float32)
    bh = np.asarray(bh, dtype=np.float32)

    x8 = np.ascontiguousarray(x.transpose(0, 2, 1)).astype(F8NP)        # (B,D,S)
    wz0 = np.ascontiguousarray(Wz[0].T).astype(F8NP)                     # (D,H)
    wh0 = np.ascontiguousarray(Wh[0].T).astype(F8NP)
    wz1 = np.ascontiguousarray(Wz[1].T).astype(F8NP)
    wh1 = np.ascontiguousarray(Wh[1].T).astype(np.float16)

    def tile_bias(b):  # (L,H) -> (L,P,HT) with [l,p,ht] = b[l, ht*P + p]
        return np.ascontiguousarray(
            b.reshape(L, HT, P).transpose(0, 2, 1)
        ).astype(np.float32)

    biases = np.ascontiguousarray(
        np.stack(
            [tile_bias(bz), tile_bias(-bz), tile_bias(bh), tile_bias(bh + 0.5)],
            axis=1,
        )
    )  # (L, 4, P, HT)

    return [
        {"x8": x8[b], "wz0": wz0, "wh0": wh0, "wz1": wz1, "wh1": wh1,
         "biases": biases}
        for b in range(B)
    ]


def kernel(x, Wz, bz, Wh, bh, _bench_iters=0):
    global LAST_EXEC_NS, _LAST_IN_MAPS
    runner = _get_runner()
    in_maps = _preprocess(x, Wz, bz, Wh, bh)
    _LAST_IN_MAPS = in_maps
    if _bench_iters:
        LAST_EXEC_NS, results = runner.bench(in_maps, iters=_bench_iters)
    else:
        results = runner.run(in_maps)
    out = np.stack([results[b]["outT"].T for b in range(B)], axis=0)
    return np.ascontiguousarray(out.astype(np.float32))


# revision 49
# speedup vs baseline: 69.6543x; 1.0015x over previous
"""MinGRU (L=2, B=8, S=2048, D=H=1024) Trainium2 Bass kernel.

Sharding: data-parallel over batch B across the 8 NeuronCores (1 sequence
per core); weights replicated.

Mixed-precision PE plan (measured: fp8e4 DoubleRow matmul = 216ns/instr,
same as fp16, but 2x contraction per instr -> 2x PE throughput; the
candidate (Wh) paths are precision-critical, the gate (Wz) paths are not):
  layer 0: both paths fp8e4 DoubleRow          (16 DR instr / unit)
  layer 1 sb-pair 0: gate fp8e4 DR, cand. fp16 (8 DR + 16 fp16 / unit)
  layer 1 sb-pair 1: both paths fp16-rhs       (32 fp16 / unit; gate uses
    mixed fp8-weights x fp16-rhs plain matmuls -> no fp8 cast needed)
Measured rel_absmax error: 1.254% (gate 2%).

Per-core dataflow (all shapes per core):
  inputs (host-preprocessed): x8 (D,S) fp8e4, wz0/wh0/wz1 (D,H) fp8e4,
  wh1 (D,H) fp16, biases as (2,4,128,H/128) fp32 tiles (bz,-bz,bh,bh+.5).
  unit = (layer, sb-pair, ht): psum pk/pp [128,1024] (2 banks each,
  2 sb-halves of 512 accumulated separately, weights reused across the
  pair halving LDWEIGHTS). Epilogue on [128,1024]:
    ACT: z = sigmoid(k+bz), c = sigmoid(-(k+bz)), sg = sigmoid(p+bh)
    DVE: g = (p + (bh+.5)) max sg   (exact piecewise-g identity)
    Pool: v = z*g (fp16)
    DVE: h[t] = c[t]*h[t-1] + v[t], h0=0.5 (tensor_tensor_scan, fp32 state)
  layer-0 scan emits fp16 into h1 (the layer-1 fp16 rhs); an inline
  copy (ACT in the elementwise-bound phase, Pool in the PE-bound
  interleave) converts h1 -> fp8 for the layer-1 gate path. layer-1
  scan emits fp32 chunks DMA'd to DRAM as (H,S); host transposes back.
  Linear-space scan is numerically safe: all terms positive, h in
  [~1e-3, ~4], fp32 state.

Schedule: A) layer-0 sb-pair 0 alone; B/C) layer-0 sb-pair 1 interleaved
with layer-1 sb-pair 0 (PE-heavy fills the elementwise-bound idle, so
the PE p-state never down-clocks); D) layer-1 sb-pair 1. h1/h18 are
split per sb-pair (tile-granular dep tracking would otherwise stall
layer-1 reads on all layer-0 scans). Measured on HW: 191.5 us vs the
319.6 us fp16 baseline; rel_absmax err 1.254% (gate 2e-2).
"""

import os
import sys

for _p in (
    "/root/.axon_site",
    "/root/.axon_site/_ro/trn_rl_repo",
    "/root/.axon_site/_ro/pypackages",
    "/opt/trn_rl_repo",
    "/opt/pypackages",
):
    if os.path.isdir(_p) and _p not in sys.path:
        sys.path.append(_p)

from contextlib import ExitStack

import numpy as np

import concourse.bacc as bacc
import concourse.bass as bass
import concourse.tile as tile
from concourse import mybir

L, B, S, D, H = 2, 8, 2048, 1024, 1024
P = 128
DT = D // P          # 8 contraction tiles
HT = H // P          # 8 output-channel tiles
SB = 512             # matmul free size (one PSUM bank of fp32)
SBP = 2 * SB         # epilogue unit width (2 banks)
NSBP = S // SBP      # 2 sb-pairs

F16 = mybir.dt.float16
F32 = mybir.dt.float32
F8 = mybir.dt.float8e4
AF = mybir.ActivationFunctionType
OP = mybir.AluOpType
DR = mybir.MatmulPerfMode.DoubleRow

LAST_EXEC_NS = None

N_WARM = 35


def _build(reps=1, split_last=4, n_warm=N_WARM):
    nc = bacc.Bacc("TRN2", target_bir_lowering=False, debug=False)

    x8d = nc.dram_tensor("x8", (D, S), F8, kind="ExternalInput")
    wz0d = nc.dram_tensor("wz0", (D, H), F8, kind="ExternalInput")
    wh0d = nc.dram_tensor("wh0", (D, H), F8, kind="ExternalInput")
    wz1d = nc.dram_tensor("wz1", (D, H), F8, kind="ExternalInput")
    wh1d = nc.dram_tensor("wh1", (D, H), F16, kind="ExternalInput")
    # biases pre-tiled on host: [l, f, p, ht] = bias_f[l, ht*128 + p]
    # f in (bz, -bz, bh, bh+0.5)
    bias_d = nc.dram_tensor("biases", (L, 4, P, HT), F32, kind="ExternalInput")
    outT = nc.dram_tensor("outT", (H, S), F16, kind="ExternalOutput")

    with tile.TileContext(nc) as tc, ExitStack() as ctx:
        persist = ctx.enter_context(tc.tile_pool(name="persist", bufs=1))
        zpool = ctx.enter_context(tc.tile_pool(name="zs", bufs=4))
        cvpool = ctx.enter_context(tc.tile_pool(name="cv", bufs=4))
        # layer-1 output chunks: same-ht chunks are HT units apart in
        # sbp-major order; keep enough slots live for the scan chaining
        ochunk_pool = ctx.enter_context(tc.tile_pool(name="ochunk", bufs=HT + 2))
        pk_pool = ctx.enter_context(tc.tile_pool(name="pk", bufs=2, space="PSUM"))
        pp_pool = ctx.enter_context(tc.tile_pool(name="pp", bufs=2, space="PSUM"))

        # ---- persistent SBUF state ----
        # NOTE: allocation order controls SBUF addresses. h1_sb (the layer-0
        # scan destination) is placed LAST so it does not share an SBUF bank
        # with the PE-streamed tiles (x_sb/weights): concurrent PE rhs reads
        # from an adjacent bank were measured to halve DVE scan throughput.
        x_sb = persist.tile([P, DT, S], F8)        # layer-0 input
        # h1/h18 split per sb-pair: readers of sb-pair 0 must not wait on
        # the layer-0 sb-pair-1 scans (tile-granular dependency tracking)
        h18_sb = [persist.tile([P, HT, SBP], F8, name="h18_0")]  # fp8 l1 gate rhs (sb-pair 0 only)
        w_sb = {
            ("wz", 0): persist.tile([P, DT, H], F8, name="wz0_sb"),
            ("wh", 0): persist.tile([P, DT, H], F8, name="wh0_sb"),
            ("wz", 1): persist.tile([P, DT, H], F8, name="wz1_sb"),
            ("wh", 1): persist.tile([P, DT, H], F16, name="wh1_sb"),
        }
        h1_sb = [persist.tile([P, HT, SBP], F16, name=f"h1_{i}")
                 for i in range(NSBP)]             # layer-0 out = l1 fp16 rhs
        w_dram = {("wz", 0): wz0d, ("wh", 0): wh0d,
                  ("wz", 1): wz1d, ("wh", 1): wh1d}
        bias_tiles = [
            persist.tile([P, 4, HT], F32, name=f"bias{l}_sb") for l in range(L)
        ]
        bias_sb = {}
        for l in range(L):
            for fi, nm in enumerate(("bz", "bzn", "bh", "bh05")):
                bias_sb[(nm, l)] = bias_tiles[l][:, fi]

        def load_w(nm, l, h0, h1):
            src = w_dram[(nm, l)].rearrange("(dt p) h -> p dt h", p=P)
            nc.sync.dma_start(out=w_sb[(nm, l)][:, :, h0:h1], in_=src[:, :, h0:h1])

        x_r = x8d.rearrange("(dt p) s -> p dt s", p=P)

        def load_x(sb):
            nc.sync.dma_start(
                out=x_sb[:, :, sb * SB : (sb + 1) * SB],
                in_=x_r[:, :, sb * SB : (sb + 1) * SB],
            )

        # PE warmup: dummy matmuls on a zeroed tile run during the DMA
        # lead-in so the HAM clock gate reaches 2.4 GHz before real work.
        warm = persist.tile([P, SB], F16, name="warm")
        nc.gpsimd.memset(warm, 0.0)
        warm_ps = pk_pool.tile([P, SBP], F32, name="pk")
        for _ in range(n_warm):
            nc.tensor.matmul(warm_ps[:, :SB], warm[:, :P], warm, start=True, stop=True)

        # DMA emission in first-consumption order (per-DMA queue overhead
        # is significant, batch where possible).
        load_w("wz", 0, 0, P)        # first unit's gate weights (0.125 MB)
        load_w("wh", 0, 0, P)
        load_x(0)
        load_x(1)
        load_w("wz", 0, P, H)
        load_w("wh", 0, P, H)
        nc.sync.dma_start(
            out=bias_tiles[0], in_=bias_d[0].rearrange("f p ht -> p f ht")
        )
        load_x(2)
        load_x(3)
        nc.sync.dma_start(
            out=bias_tiles[1], in_=bias_d[1].rearrange("f p ht -> p f ht")
        )
        for half in range(2):
            load_w("wz", 1, half * (H // 2), (half + 1) * (H // 2))
            load_w("wh", 1, half * (H // 2), (half + 1) * (H // 2))

        def mm_group_f8dr(ps, wmat, rhs8, h0, h1, s0):
            # K=1024 via 4 DoubleRow instrs per sb-half; weights reused
            # across the two halves (LDWEIGHTS once per j, 2 matmuls)
            for j in range(DT // 2):
                for sl in range(2):
                    nc.tensor.matmul(
                        ps[:, sl * SB : (sl + 1) * SB],
                        wmat[:, 2 * j : 2 * j + 2, h0:h1],
                        rhs8[:, 2 * j : 2 * j + 2, s0 + sl * SB : s0 + (sl + 1) * SB],
                        start=(j == 0),
                        stop=(j == DT // 2 - 1),
                        perf_mode=DR,
                        skip_group_check=True,
                    )

        def mm_group_f16(ps, wmat, rhs16, h0, h1, s0):
            for dt_i in range(DT):
                for sl in range(2):
                    nc.tensor.matmul(
                        ps[:, sl * SB : (sl + 1) * SB],
                        wmat[:, dt_i, h0:h1],
                        rhs16[:, dt_i, s0 + sl * SB : s0 + (sl + 1) * SB],
                        start=(dt_i == 0),
                        stop=(dt_i == DT - 1),
                        skip_group_check=True,
                    )

        def mm_group_f16_half(ps, wmat, rhs16, h0, h1, s0, sl):
            for dt_i in range(DT):
                nc.tensor.matmul(
                    ps[:, sl * SB : (sl + 1) * SB],
                    wmat[:, dt_i, h0:h1],
                    rhs16[:, dt_i, s0 + sl * SB : s0 + (sl + 1) * SB],
                    start=(dt_i == 0),
                    stop=(dt_i == DT - 1),
                    skip_group_check=True,
                )

        # deferred h1 -> fp8 casts: emitted off the critical chain, spread
        # over later units on whichever engine has slack in that phase
        cast_queue = []
        cast_rr = [0]

        def drain_casts(engines, n=1, min_queue=0):
            for _ in range(n):
                if len(cast_queue) <= min_queue:
                    return
                ht, col0, w = cast_queue.pop(0)
                sp, c0 = col0 // SBP, col0 % SBP
                dst = h18_sb[sp][:, ht, c0 : c0 + w]
                src = h1_sb[sp][:, ht, c0 : c0 + w]
                eng = engines[cast_rr[0] % len(engines)]
                cast_rr[0] += 1
                if eng == "act":
                    nc.scalar.activation(dst, src, AF.Copy)
                elif eng == "pool":
                    nc.gpsimd.tensor_copy(dst, src)
                else:
                    nc.vector.tensor_copy(dst, src)

        def emit_unit(l, sbp, ht, out_writer, drain_n=1):
            wz = w_sb[("wz", l)]
            wh = w_sb[("wh", l)]
            bz_t = bias_sb[("bz", l)]
            bzn_t = bias_sb[("bzn", l)]
            bh_t = bias_sb[("bh", l)]
            bh05_t = bias_sb[("bh05", l)]
            s0 = sbp * SBP
            if True:
                if True:
                    # casts ahead of this unit's matmuls: layer-1 gate
                    # matmuls consume h18, so finish pending chunks early.
                    # NOT on DVE: its sem orders the h18-read waits, and a
                    # lagging DVE queue stalls every sb-pair-1 unit's ldw.
                    if l == 1:
                        drain_casts(("act", "pool"), n=drain_n)
                    h0, h1 = ht * P, (ht + 1) * P
                    pk = pk_pool.tile([P, SBP], F32, name="pk")
                    pp = pp_pool.tile([P, SBP], F32, name="pp")
                    last = l == 1 and sbp == NSBP - 1 and ht >= HT - 3
                    # cand. path first: sg/g depend only on p, so they
                    # overlap the gate path's matmuls. The very last unit
                    # runs the gate path FIRST so z/c compute during the
                    # cand. matmuls (short tail).
                    # layer-1 sb-pair 1 reads the gate rhs in fp16 (mixed
                    # fp8-weights x fp16-rhs plain matmuls): the fp8 casts
                    # for that half ran on a saturated Pool queue and their
                    # backlog cascaded into a ~30us post-matmul tail.
                    if l == 0:
                        mm_group_f8dr(pp, wh, x_sb, h0, h1, s0)
                        mm_group_f8dr(pk, wz, x_sb, h0, h1, s0)
                    elif last:
                        mm_group_f16(pk, wz, h1_sb[sbp], h0, h1, 0)
                        mm_group_f16(pp, wh, h1_sb[sbp], h0, h1, 0)
                    elif sbp == NSBP - 1:
                        mm_group_f16(pp, wh, h1_sb[sbp], h0, h1, 0)
                        mm_group_f16(pk, wz, h1_sb[sbp], h0, h1, 0)
                    else:
                        mm_group_f16(pp, wh, h1_sb[sbp], h0, h1, 0)
                        mm_group_f8dr(pk, wz, h18_sb[sbp], h0, h1, 0)
                    # fast-drain epilogue for the kernel tail: z/c full-width
                    # up front (pk group ran first), then fine chunks with
                    # everything after ACT on DVE (fewer cross-engine hops)
                    sub = split_last if last else 1
                    w = SBP // sub
                    z_full = cc_full = None
                    if last:
                        z_full = zpool.tile([P, SBP], F16, name="z")
                        cc_full = cvpool.tile([P, SBP], F16, name="cc")
                        nc.scalar.activation(
                            z_full, pk, AF.Sigmoid,
                            bias=bz_t[:, ht : ht + 1], scale=1.0,
                        )
                        nc.scalar.activation(
                            cc_full, pk, AF.Sigmoid,
                            bias=bzn_t[:, ht : ht + 1], scale=-1.0,
                        )
                    for si in range(sub):
                        c0 = si * w
                        sg = zpool.tile([P, SBP], F16, name="sg")[:, :w]
                        # ACT order matches the dependency chain: g needs sg
                        # first, the scan needs c last
                        nc.scalar.activation(
                            sg, pp[:, c0 : c0 + w], AF.Sigmoid,
                            bias=bh_t[:, ht : ht + 1], scale=1.0,
                        )
                        g = zpool.tile([P, SBP], F16, name="g")[:, :w]
                        # g = (p + (bh+0.5)) max sg. Layer 1 routes p+bh05
                        # through ACT so the DVE op is a short all-SBUF fp16
                        # max: the DVE stt was the last PSUM reader, and its
                        # latency behind the scan stalls the unit+2 matmuls
                        # (PSUM double-buffer WAR).
                        if l == 1 and not last:
                            pl = zpool.tile([P, SBP], F16, name="pl")[:, :w]
                            nc.scalar.activation(
                                pl, pp[:, c0 : c0 + w], AF.Identity,
                                bias=bh05_t[:, ht : ht + 1], scale=1.0,
                            )
                            nc.vector.tensor_max(g, pl, sg)
                        else:
                            nc.vector.scalar_tensor_tensor(
                                g, pp[:, c0 : c0 + w], bh05_t[:, ht : ht + 1],
                                sg, op0=OP.add, op1=OP.max,
                            )
                        if last:
                            z = z_full[:, c0 : c0 + w]
                            cc = cc_full[:, c0 : c0 + w]
                            v = cvpool.tile([P, SBP], F16, name="v")[:, :w]
                            nc.vector.tensor_mul(v, z, g)
                        else:
                            z = zpool.tile([P, SBP], F16, name="z")
                            cc = cvpool.tile([P, SBP], F16, name="cc")
                            v = cvpool.tile([P, SBP], F16, name="v")
                            nc.scalar.activation(
                                z, pk, AF.Sigmoid,
                                bias=bz_t[:, ht : ht + 1], scale=1.0,
                            )
                            nc.scalar.activation(
                                cc, pk, AF.Sigmoid,
                                bias=bzn_t[:, ht : ht + 1], scale=-1.0,
                            )
                            nc.gpsimd.tensor_mul(v, z, g)
                        out_writer(ht, s0 + c0, w, cc, v)
                    # during layer 0, drain casts one unit-sweep late so the
                    # cast never sits between v's on the Pool queue
                    if l == 0 and drain_n:
                        drain_casts(("act", "pool"), n=drain_n, min_queue=HT)

        # layer 0: scan into h1_sb (f16), chained over blocks; the fp8
        # convert is emitted inline right after each scan, on the engine
        # with slack in that phase (ACT during sb-pair 0's elementwise-bound
        # stretch, Pool during the PE-bound interleave) — batching casts at
        # phase boundaries gated the next phase's matmuls for ~5us each.
        def l0_writer(ht, col0, w, cc, v):
            sp, c0 = col0 // SBP, col0 % SBP
            dst = h1_sb[sp][:, ht, c0 : c0 + w]
            if col0 == 0:
                init = 0.5
            elif c0 == 0:
                init = h1_sb[sp - 1][:, ht, SBP - 1 : SBP]
            else:
                init = h1_sb[sp][:, ht, c0 - 1 : c0]
            nc.vector.tensor_tensor_scan(dst, cc, v, init, op0=OP.mult, op1=OP.add)
            if sp == 0:
                # only sb-pair 0 needs the fp8 copy (layer-1 DR gate path);
                # sb-pair 1's gate matmuls read h1 fp16 directly
                dst8 = h18_sb[0][:, ht, c0 : c0 + w]
                nc.scalar.activation(dst8, dst, AF.Copy)

        # layer 1: scan into fp32 chunks, DMA out per chunk
        prev_chunk = {}

        def l1_writer(ht, col0, w, cc, v):
            oc = ochunk_pool.tile([P, SBP], F16, name="oc")[:, :w]
            if col0 == 0:
                init = 0.5
            else:
                pt, pw = prev_chunk[ht]
                init = pt[:, pw - 1 : pw]
            nc.vector.tensor_tensor_scan(oc, cc, v, init, op0=OP.mult, op1=OP.add)
            prev_chunk[ht] = (oc, w)
            nc.sync.dma_start(out=outT[ht * P : (ht + 1) * P, col0 : col0 + w], in_=oc)

        def body():
            # Phase A: layer-0 sb-pair 0 alone (elementwise-bound).
            for ht in range(HT):
                emit_unit(0, 0, ht, l0_writer, drain_n=0)
            # Phase B/C: interleave layer-0 sb-pair-1 units (elementwise-
            # heavy, PE-light) with layer-1 sb-pair-0 units (PE-heavy) so
            # the PE never idles long enough to down-clock.
            seq = [(0, 1, 0), (0, 1, 1)]
            for i in range(2, HT):
                seq.append((0, 1, i))
                seq.append((1, 0, i - 2))
            seq += [(1, 0, HT - 2), (1, 0, HT - 1)]
            for l, sbp, ht in seq:
                emit_unit(l, sbp, ht, l0_writer if l == 0 else l1_writer,
                          drain_n=0)
            # Phase D: layer-1 sb-pair 1 (PE-bound)
            for ht in range(HT):
                emit_unit(1, 1, ht, l1_writer, drain_n=0)

        if reps == 1:
            body()
        else:
            with tc.For_i(0, reps, 1, hint_engines=tuple(nc.engines)):
                body()

    nc.finalize()
    return nc


class _Runner:
    """Compile the bass module once into a jitted shard_map over 8 cores."""

    def __init__(self, reps=1, **build_kwargs):
        import jax
        from jax.experimental.shard_map import shard_map
        from jax.sharding import Mesh, NamedSharding, PartitionSpec

        from concourse import bass2jax, mybir as _mybir

        self.jax = jax
        nc = _build(reps, **build_kwargs)
        self.nc = nc
        bass2jax.install_neuronx_cc_hook()

        partition_name = (
            nc.partition_id_tensor.name if nc.partition_id_tensor else None
        )
        in_names, out_names, out_avals, zero_shapes = [], [], [], []
        for alloc in nc.m.functions[0].allocations:
            if not isinstance(_mybir.MemoryLocationSet, type) or not isinstance(
                alloc, _mybir.MemoryLocationSet
            ):
                continue
            name = alloc.memorylocations[0].name
            if alloc.kind == "ExternalInput":
                if name != partition_name:
                    in_names.append(name)
            elif alloc.kind == "ExternalOutput":
                shape = tuple(alloc.tensor_shape)
                dtype = _mybir.dt.np(alloc.dtype)
                out_names.append(name)
                out_avals.append(jax.core.ShapedArray(shape, dtype))
                zero_shapes.append((shape, dtype))
        self.in_names = list(in_names)
        self.out_names = out_names
        self.zero_shapes = zero_shapes
        n_params = len(in_names)
        n_outs = len(out_names)
        all_in_names = in_names + out_names
        if partition_name is not None:
            all_in_names.append(partition_name)
        donate = tuple(range(n_params, n_params + n_outs))

        def _body(*args):
            operands = list(args)
            if partition_name is not None:
                operands.append(bass2jax.partition_id_tensor())
            outs = bass2jax._bass_exec_p.bind(
                *operands,
                out_avals=tuple(out_avals),
                in_names=tuple(all_in_names),
                out_names=tuple(out_names),
                lowering_input_output_aliases=(),
                sim_require_finite=True,
                sim_require_nnan=True,
                nc=nc,
            )
            return tuple(outs)

        self._base_body = _body
        devices = jax.devices()[:B]
        assert len(devices) == B
        self.mesh = Mesh(np.asarray(devices), ("core",))
        self.sharding = NamedSharding(self.mesh, PartitionSpec("core"))
        in_specs = (PartitionSpec("core"),) * (n_params + n_outs)
        out_specs = (PartitionSpec("core"),) * n_outs
        _mapped = shard_map(
            _body,
            mesh=self.mesh,
            in_specs=in_specs,
            out_specs=out_specs,
            check_rep=False,
        )
        self.fn = jax.jit(_mapped, donate_argnums=donate, keep_unused=True)
        self.fn_nodonate = jax.jit(_mapped, keep_unused=True)

    def _concat_inputs(self, in_maps):
        return [
            np.concatenate([np.asarray(m[name]) for m in in_maps], axis=0)
            for name in self.in_names
        ]

    def _zeros(self):
        return [
            np.zeros((B * s[0], *s[1:]), dt) for (s, dt) in self.zero_shapes
        ]

    def run(self, in_maps):
        out_arrs = self.fn(*self._concat_inputs(in_maps), *self._zeros())
        return [
            {
                name: np.asarray(out_arrs[i]).reshape(B, -1, *out_arrs[i].shape[1:])[c]
                for i, name in enumerate(self.out_names)
            }
            for c in range(B)
        ]

    def bench(self, in_maps, iters=8):
        """Return (est_ns_per_iter, results_of_last)."""
        import time as _time

        jax = self.jax
        dev_in = [
            jax.device_put(a, self.sharding) for a in self._concat_inputs(in_maps)
        ]
        zero_sets = [
            [jax.device_put(z, self.sharding) for z in self._zeros()]
            for _ in range(iters + 1)
        ]
        out = self.fn(*dev_in, *zero_sets[0])  # warmup
        jax.block_until_ready(out)
        t0 = _time.perf_counter()
        for i in range(iters):
            out = self.fn(*dev_in, *zero_sets[i + 1])
        jax.block_until_ready(out)
        t1 = _time.perf_counter()
        est_ns = (t1 - t0) / iters * 1e9
        results = [
            {
                name: np.asarray(out[i]).reshape(B, -1, *out[i].shape[1:])[c]
                for i, name in enumerate(self.out_names)
            }
            for c in range(B)
        ]
        return est_ns, results


_RUNNER = None
_LAST_IN_MAPS = None


def _get_runner():
    global _RUNNER
    if _RUNNER is None:
        _RUNNER = _Runner()
    return _RUNNER


def _preprocess(x, Wz, bz, Wh, bh):
    import ml_dtypes

    F8NP = ml_dtypes.float8_e4m3

    x = np.asarray(x, dtype=np.float32)
    Wz = np.asarray(Wz, dtype=np.float32)
    bz = np.asarray(bz, dtype=np.float32)
    Wh = np.asarray(Wh, dtype=np.float32)
    bh = np.asarray(bh, dtype=np.float32)

    x8 = np.ascontiguousarray(x.transpose(0, 2, 1)).astype(F8NP)        # (B,D,S)
    wz0 = np.ascontiguousarray(Wz[0].T).astype(F8NP)                     # (D,H)
    wh0 = np.ascontiguousarray(Wh[0].T).astype(F8NP)
    wz1 = np.ascontiguousarray(Wz[1].T).astype(F8NP)
    wh1 = np.ascontiguousarray(Wh[1].T).astype(np.float16)

    def tile_bias(b):  # (L,H) -> (L,P,HT) with [l,p,ht] = b[l, ht*P + p]
        return np.ascontiguousarray(
            b.reshape(L, HT, P).transpose(0, 2, 1)
        ).astype(np.float32)

    biases = np.ascontiguousarray(
        np.stack(
            [tile_bias(bz), tile_bias(-bz), tile_bias(bh), tile_bias(bh + 0.5)],
            axis=1,
        )
    )  # (L, 4, P, HT)

    return [
        {"x8": x8[b], "wz0": wz0, "wh0": wh0, "wz1": wz1, "wh1": wh1,
         "biases": biases}
        for b in range(B)
    ]


def kernel(x, Wz, bz, Wh, bh, _bench_iters=0):
    global LAST_EXEC_NS, _LAST_IN_MAPS
    runner = _get_runner()
    in_maps = _preprocess(x, Wz, bz, Wh, bh)
    _LAST_IN_MAPS = in_maps
    if _bench_iters:
        LAST_EXEC_NS, results = runner.bench(in_maps, iters=_bench_iters)
    else:
        results = runner.run(in_maps)
    out = np.stack([results[b]["outT"].T for b in range(B)], axis=0)
    return np.ascontiguousarray(out.astype(np.float32))
